# revision 1
# baseline (speedup 1.0000x reference)
"""Trainium2 Bass kernel for nn_CotLayer (CoT attention layer).

Strategy: H-dimension sharding across 8 NeuronCores (12 of 96 rows each, all
B*T frames per core, halo rows included host-side).  All 3x3x3 convs are
implicit GEMM: channels on partitions, padded-width (98) pixel lines on the
free dim, 27 shifted matmuls accumulating in PSUM (fp32r = full PE rate).
GroupNorm statistics and the split-attention gap are AllReduced across cores.

Self-contained: only numpy/jax/concourse imports.
"""
from contextlib import ExitStack

import numpy as np

import concourse.bacc as bacc
import concourse.tile as tile
from concourse import mybir

F32 = mybir.dt.float32
F32R = mybir.dt.float32r
AF = mybir.ActivationFunctionType
ALU = mybir.AluOpType

# problem constants
B, DIM, T, H, W = 2, 128, 8, 96, 96
K, SHARE, RADIX = 3, 8, 2
EMB = 144
EPS = 1e-5
NCORES = 8
F = B * T               # 16 frames
ROWN = H // NCORES      # 12 owned rows per core
WP = W + 2              # padded width

# slab geometry (rows, line length)
RX, RKF, RE1, RWV, RXQ = 18, 16, 14, 12, 14
LX, LKF, LE1, LWV, LXQ = RX * WP, RKF * WP, RE1 * WP, RWV * WP, RXQ * WP

GN_CNT = float((EMB // 16) * T * H * W)   # 9*8*96*96 per (b, group)


def _chunks(L, n):
    # fp32r matmul requires even moving-operand counts -> even chunk sizes
    assert L % 2 == 0
    base2 = (L // n) // 2 * 2
    rem = L - base2 * n
    out, s = [], 0
    for i in range(n):
        e = s + base2 + (2 if i < rem // 2 else 0)
        out.append((s, e))
        s = e
    assert s == L
    return out


CH_KF = _chunks(LKF, 4)
CH_E1 = _chunks(LE1, 3)
CH_WV = _chunks(LWV, 3)
CH_XQ = _chunks(LXQ, 3)


def _same_batch(f, g):
    return 0 <= g < F and g // T == f // T


def build_nc(debug=False):
    nc = bacc.Bacc()

    # ---------------- I/O ----------------
    xin = nc.dram_tensor("xin", [F, DIM, LX], F32R, kind="ExternalInput")
    kew = nc.dram_tensor("kew", [DIM, 27 * DIM], F32R, kind="ExternalInput")
    keb = nc.dram_tensor("keb", [DIM, 1], F32, kind="ExternalInput")
    e1w = nc.dram_tensor("e1w", [DIM, 54 * 64], F32R, kind="ExternalInput")
    e1b = nc.dram_tensor("e1b", [64, 1], F32, kind="ExternalInput")
    e2w = nc.dram_tensor("e2w", [64, 27 * EMB], F32R, kind="ExternalInput")
    e2b = nc.dram_tensor("e2b", [EMB, 1], F32, kind="ExternalInput")
    c1w = nc.dram_tensor("c1w", [DIM, 27 * DIM], F32R, kind="ExternalInput")
    c1b = nc.dram_tensor("c1b", [DIM, 1], F32, kind="ExternalInput")
    bn2s = nc.dram_tensor("bn2s", [DIM, 2], F32, kind="ExternalInput")
    se1w_d = nc.dram_tensor("se1w", [DIM, 3 * 64], F32R, kind="ExternalInput")
    se1b_d = nc.dram_tensor("se1b", [64, 1], F32, kind="ExternalInput")
    se2w_d = nc.dram_tensor("se2w", [64, 3 * 256], F32R, kind="ExternalInput")
    se2bd_d = nc.dram_tensor("se2bd", [DIM, 1], F32, kind="ExternalInput")
    gng_d = nc.dram_tensor("gng", [16, 9], F32, kind="ExternalInput")
    gnb_d = nc.dram_tensor("gnb", [16, 9], F32, kind="ExternalInput")
    E_d = nc.dram_tensor("Emat", [16, DIM], F32R, kind="ExternalInput")
    G_d = nc.dram_tensor("Gmat", [DIM, 16], F32, kind="ExternalInput")
    Gh_d = nc.dram_tensor("Ghmat", [16, 16], F32, kind="ExternalInput")
    kmask_d = nc.dram_tensor("kmask", [DIM, LKF], F32, kind="ExternalInput")
    e1mask_d = nc.dram_tensor("e1mask", [64, LE1], F32, kind="ExternalInput")
    wvmask_d = nc.dram_tensor("wvmask", [DIM, LWV], F32, kind="ExternalInput")
    xqmask_d = nc.dram_tensor("xqmask", [DIM, LXQ], F32, kind="ExternalInput")

    out_d = nc.dram_tensor("out", [B, DIM, T, ROWN, W], F32, kind="ExternalOutput")

    dbg = "ExternalOutput" if debug else "Internal"
    wv_dram = nc.dram_tensor("wv_s", [F, EMB, LWV], F32R, kind=dbg)
    xq_dram = nc.dram_tensor("xq_s", [F, DIM, LXQ], F32, kind=dbg)
    kf_dram = nc.dram_tensor("kf_s", [F, DIM, LWV], F32, kind=dbg)
    y_dram = nc.dram_tensor("y_s", [F, DIM, LWV], F32, kind=dbg)
    if debug:
        attn_dbg = nc.dram_tensor("attn_dbg", [DIM, 16], F32, kind="ExternalOutput")
        gstat_dbg = nc.dram_tensor("gstat_dbg", [16, 4], F32, kind="ExternalOutput")

    ar1_in = nc.dram_tensor("ar1_in", [16, 4], F32)
    ar1_out = nc.dram_tensor("ar1_out", [16, 4], F32, addr_space="Shared")
    ar2_in = nc.dram_tensor("ar2_in", [DIM, F], F32)
    ar2_out = nc.dram_tensor("ar2_out", [DIM, F], F32, addr_space="Shared")
    RG = [list(range(NCORES))]

    with tile.TileContext(nc) as tc, ExitStack() as stk:
        consts = stk.enter_context(tc.tile_pool(name="consts", bufs=1))

        def load_const(dram, p, l, dt=F32):
            t = consts.tile([p, l], dt, name=dram.name + "_sb")
            nc.sync.dma_start(t[:], dram[:, :])
            return t

        kew_sb = load_const(kew, DIM, 27 * DIM, F32R)
        e1w_sb = load_const(e1w, DIM, 54 * 64, F32R)
        e2w_sb = load_const(e2w, 64, 27 * EMB, F32R)
        c1w_sb = load_const(c1w, DIM, 27 * DIM, F32R)
        se1w_sb = load_const(se1w_d, DIM, 3 * 64, F32R)
        se2w_sb = load_const(se2w_d, 64, 3 * 256, F32R)
        E_sb = load_const(E_d, 16, DIM, F32R)
        G_sb = load_const(G_d, DIM, 16)
        Gh_sb = load_const(Gh_d, 16, 16)
        keb_sb = load_const(keb, DIM, 1)
        e1b_sb = load_const(e1b, 64, 1)
        c1b_sb = load_const(c1b, DIM, 1)
        bn2_sb = load_const(bn2s, DIM, 2)
        se1b_sb = load_const(se1b_d, 64, 1)
        se2bd_sb = load_const(se2bd_d, DIM, 1)
        gng_sb = load_const(gng_d, 16, 9)
        gnb_sb = load_const(gnb_d, 16, 9)
        wvmask_sb = load_const(wvmask_d, DIM, LWV)
        e2b_lo_sb = consts.tile([DIM, 1], F32)
        nc.sync.dma_start(e2b_lo_sb[:], e2b[0:DIM, :])
        e2b_hi_sb = consts.tile([16, 1], F32)
        nc.sync.dma_start(e2b_hi_sb[:], e2b[DIM:EMB, :])

        ztiny = consts.tile([DIM, 24], F32)
        nc.vector.memset(ztiny[:], 0.0)

        # stats / gap accumulators
        ssum_lo = consts.tile([DIM, 3 * F], F32)
        ssq_lo = consts.tile([DIM, 3 * F], F32)
        ssum_hi = consts.tile([16, 3 * F], F32)
        ssq_hi = consts.tile([16, 3 * F], F32)
        gap_sb = consts.tile([DIM, F], F32)
        gapy_sb = consts.tile([DIM, F], F32)
        gstat_sb = consts.tile([16, 4], F32)
        scale_sb = consts.tile([16, 2 * 9], F32)
        bias_sb = consts.tile([16, 2 * 9], F32)
        attn0_sb = consts.tile([DIM, F], F32)
        sred_lo = consts.tile([DIM, 4], F32)
        sred_hi = consts.tile([16, 4], F32)

        # =========================================================
        # PHASE 1
        # =========================================================
        stk1 = ExitStack()
        p1 = stk1.enter_context(tc.tile_pool(name="p1", bufs=1))
        p1stage = stk1.enter_context(tc.tile_pool(name="p1stage", bufs=2))
        stk1ps = ExitStack()
        ps_ke = stk1ps.enter_context(tc.tile_pool(name="ps_ke", bufs=2, space="PSUM"))
        ps_e1 = stk1ps.enter_context(tc.tile_pool(name="ps_e1", bufs=2, space="PSUM"))
        ps_e2l = stk1ps.enter_context(tc.tile_pool(name="ps_e2l", bufs=2, space="PSUM"))
        ps_e2h = stk1ps.enter_context(tc.tile_pool(name="ps_e2h", bufs=2, space="PSUM"))

        kmask_sb = p1.tile([DIM, LKF], F32)
        nc.sync.dma_start(kmask_sb[:], kmask_d[:, :])
        e1mask_sb = p1.tile([64, LE1], F32)
        nc.sync.dma_start(e1mask_sb[:], e1mask_d[:, :])
        xqmask_sb = p1.tile([DIM, LXQ], F32)
        nc.sync.dma_start(xqmask_sb[:], xqmask_d[:, :])

        zslab = p1.tile([DIM, LX + 2], F32R)
        zsf = p1.tile([DIM, LX + 2], F32)
        nc.vector.memset(zsf[:], 0.0)
        nc.vector.tensor_copy(zslab[:], zsf[:])

        x_ring = [p1.tile([DIM, LX + 2], F32R, tag=f"xr{i}", name=f"xr{i}")
                  for i in range(4)]
        kf_ring = [p1.tile([DIM, LKF + 2], F32R, tag=f"kfr{i}", name=f"kfr{i}")
                   for i in range(3)]
        e1_ring = [p1.tile([64, LE1 + 2], F32R, tag=f"e1r{i}", name=f"e1r{i}")
                   for i in range(3)]
        for t_ in x_ring + kf_ring + e1_ring:
            pp = t_.shape[0]
            L = t_.shape[1]
            nc.vector.tensor_copy(t_[:, 0:1], zslab[:pp, 0:1])
            nc.vector.tensor_copy(t_[:, L - 1:L], zslab[:pp, 0:1])

        def load_x(f):
            nc.sync.dma_start(x_ring[f % 4][:, 1:LX + 1], xin[f])

        def xsrc(f, d):
            return x_ring[(f + d) % 4] if _same_batch(f, f + d) else zslab

        def kfsrc(f, d):
            return kf_ring[(f + d) % 3] if _same_batch(f, f + d) else zslab

        def e1src(f, d):
            return e1_ring[(f + d) % 3] if _same_batch(f, f + d) else zslab

        # ---------------- PHASE 1a ----------------
        load_x(0)
        for i in range(F + 2):
            if i + 1 < F:
                load_x(i + 1)
            # A: kf[i]
            if i < F:
                kf_t = kf_ring[i % 3]
                for (q0, q1) in CH_KF:
                    n = q1 - q0
                    p = ps_ke.tile([DIM, n], F32, tag="ke")
                    for kt in range(K):
                        src = xsrc(i, kt - 1)
                        for kh in range(K):
                            for kw in range(K):
                                tap = (kt * K + kh) * K + kw
                                off = 1 + q0 + kh * WP + kw - 1
                                nc.tensor.matmul(
                                    p[:], kew_sb[:, tap * DIM:(tap + 1) * DIM],
                                    src[:, off:off + n],
                                    start=(tap == 0), stop=(tap == 26))
                    nc.scalar.activation(kf_t[:, 1 + q0:1 + q1], p[:], AF.Relu,
                                         bias=keb_sb[:, 0:1], scale=1.0)
                nc.vector.tensor_mul(kf_t[:, 1:LKF + 1], kf_t[:, 1:LKF + 1],
                                     kmask_sb[:])
                kstore = p1stage.tile([DIM, LWV], F32, tag="kfstore")
                own = kf_t[:, 1:LKF + 1].rearrange("p (r w) -> p r w", r=RKF)[:, 2:14, :]
                nc.scalar.activation(
                    kstore[:].rearrange("p (r w) -> p r w", r=RWV), own,
                    AF.Identity, bias=0.0, scale=1.0,
                    accum_out=gap_sb[:, i:i + 1])
                nc.sync.dma_start(kf_dram[i], kstore[:])
            # B: e1out[i-1] (before x[i+1] prefetch overwrites x[i-2])
            tb = i - 1
            if 0 <= tb < F:
                e1_t = e1_ring[tb % 3]
                for (q0, q1) in CH_E1:
                    n = q1 - q0
                    p = ps_e1.tile([64, n], F32, tag="e1")
                    for kt in range(K):
                        sx = xsrc(tb, kt - 1)
                        sk = kfsrc(tb, kt - 1)
                        for kh in range(K):
                            for kw in range(K):
                                tap = (kt * K + kh) * K + kw
                                offx = 1 + q0 + (kh + 1) * WP + kw - 1
                                offk = 1 + q0 + kh * WP + kw - 1
                                c0 = (tap * 2) * 64
                                nc.tensor.matmul(
                                    p[:], e1w_sb[:, c0:c0 + 64],
                                    sx[:, offx:offx + n],
                                    start=(tap == 0), stop=False)
                                nc.tensor.matmul(
                                    p[:], e1w_sb[:, c0 + 64:c0 + 128],
                                    sk[:, offk:offk + n],
                                    start=False, stop=(tap == 26))
                    nc.scalar.activation(e1_t[:, 1 + q0:1 + q1], p[:], AF.Relu,
                                         bias=e1b_sb[:, 0:1], scale=1.0)
                nc.vector.tensor_mul(e1_t[:, 1:LE1 + 1], e1_t[:, 1:LE1 + 1],
                                     e1mask_sb[:])
            # C: wv[i-2]
            f2 = i - 2
            if 0 <= f2 < F:
                wlo = p1stage.tile([DIM, LWV], F32R, tag="wvlo")
                whi = p1stage.tile([16, LWV], F32R, tag="wvhi")
                sqj = p1stage.tile([DIM, 392], F32, tag="sqjunk")
                for ci, (q0, q1) in enumerate(CH_WV):
                    n = q1 - q0
                    plo = ps_e2l.tile([DIM, n], F32, tag="e2l")
                    phi = ps_e2h.tile([16, n], F32, tag="e2h")
                    for kt in range(K):
                        se = e1src(f2, kt - 1)
                        for kh in range(K):
                            for kw in range(K):
                                tap = (kt * K + kh) * K + kw
                                off = 1 + q0 + kh * WP + kw - 1
                                c0 = tap * EMB
                                nc.tensor.matmul(
                                    plo[:], e2w_sb[:, c0:c0 + DIM],
                                    se[:64, off:off + n],
                                    start=(tap == 0), stop=(tap == 26))
                                nc.tensor.matmul(
                                    phi[:], e2w_sb[:, c0 + DIM:c0 + EMB],
                                    se[:64, off:off + n],
                                    start=(tap == 0), stop=(tap == 26))
                    col = f2 * 3 + ci
                    nc.vector.scalar_tensor_tensor(
                        wlo[:, q0:q1], plo[:], e2b_lo_sb[:, 0:1],
                        wvmask_sb[:, q0:q1],
                        ALU.add, ALU.mult, accum_out=ssum_lo[:, col:col + 1])
                    nc.vector.scalar_tensor_tensor(
                        whi[:, q0:q1], phi[:], e2b_hi_sb[:, 0:1],
                        wvmask_sb[:16, q0:q1],
                        ALU.add, ALU.mult, accum_out=ssum_hi[:, col:col + 1])
                    nc.scalar.activation(sqj[:, :n], wlo[:, q0:q1], AF.Square,
                                         bias=0.0, scale=1.0,
                                         accum_out=ssq_lo[:, col:col + 1])
                    nc.scalar.activation(sqj[:16, :n], whi[:, q0:q1], AF.Square,
                                         bias=0.0, scale=1.0,
                                         accum_out=ssq_hi[:, col:col + 1])
                nc.sync.dma_start(wv_dram[f2, 0:DIM], wlo[:])
                nc.sync.dma_start(wv_dram[f2, DIM:EMB], whi[:])

        # ---- GN stats reduce + AllReduce ----
        stk1ps.close()
        half = 3 * T
        nc.vector.reduce_sum(sred_lo[:, 0:1], ssum_lo[:, 0:half], axis=mybir.AxisListType.X)
        nc.vector.reduce_sum(sred_lo[:, 1:2], ssum_lo[:, half:], axis=mybir.AxisListType.X)
        nc.vector.reduce_sum(sred_lo[:, 2:3], ssq_lo[:, 0:half], axis=mybir.AxisListType.X)
        nc.vector.reduce_sum(sred_lo[:, 3:4], ssq_lo[:, half:], axis=mybir.AxisListType.X)
        nc.vector.reduce_sum(sred_hi[:, 0:1], ssum_hi[:, 0:half], axis=mybir.AxisListType.X)
        nc.vector.reduce_sum(sred_hi[:, 1:2], ssum_hi[:, half:], axis=mybir.AxisListType.X)
        nc.vector.reduce_sum(sred_hi[:, 2:3], ssq_hi[:, 0:half], axis=mybir.AxisListType.X)
        nc.vector.reduce_sum(sred_hi[:, 3:4], ssq_hi[:, half:], axis=mybir.AxisListType.X)
        with tc.tile_pool(name="ps_st", bufs=1, space="PSUM") as ps_st:
            pst = ps_st.tile([16, 4], F32, tag="gstat")
            nc.tensor.matmul(pst[:], G_sb[:], sred_lo[:], start=True, stop=False)
            nc.tensor.matmul(pst[:], Gh_sb[:], sred_hi[:], start=False, stop=True)
            gloc = consts.tile([16, 4], F32)
            nc.vector.tensor_copy(gloc[:], pst[:])
        nc.sync.dma_start(ar1_in[:, :], gloc[:])
        nc.gpsimd.collective_compute(
            "AllReduce", ALU.add, replica_groups=RG,
            ins=[ar1_in[:, :]], outs=[ar1_out[:, :]])
        nc.sync.dma_start(gstat_sb[:], ar1_out[:, :])
        if debug:
            nc.sync.dma_start(gstat_dbg[:, :], gstat_sb[:])

        # ---------------- PHASE 1b: c1 (overlaps AR1) ----------------
        with tc.tile_pool(name="ps_c1", bufs=3, space="PSUM") as ps_c1:
            load_x(0)
            for f in range(F):
                if f + 1 < F:
                    load_x(f + 1)
                xst = p1stage.tile([DIM, LXQ], F32, tag="xqstage")
                for (q0, q1) in CH_XQ:
                    n = q1 - q0
                    p = ps_c1.tile([DIM, n], F32, tag="c1")
                    for kt in range(K):
                        src = xsrc(f, kt - 1)
                        for kh in range(K):
                            for kw in range(K):
                                tap = (kt * K + kh) * K + kw
                                off = 1 + q0 + (kh + 1) * WP + kw - 1
                                nc.tensor.matmul(
                                    p[:], c1w_sb[:, tap * DIM:(tap + 1) * DIM],
                                    src[:, off:off + n],
                                    start=(tap == 0), stop=(tap == 26))
                    nc.vector.scalar_tensor_tensor(
                        xst[:, q0:q1], p[:], c1b_sb[:, 0:1], xqmask_sb[:, q0:q1],
                        ALU.add, ALU.mult)
                nc.sync.dma_start(xq_dram[f], xst[:])

        # ---- GN scale/bias ----
        mu = consts.tile([16, 2], F32)
        msq = consts.tile([16, 2], F32)
        mu2 = consts.tile([16, 2], F32)
        var = consts.tile([16, 2], F32)
        sd = consts.tile([16, 2], F32)
        rsq = consts.tile([16, 2], F32)
        tmp9 = consts.tile([16, 9], F32)
        nc.vector.tensor_scalar(mu[:], gstat_sb[:, 0:2], 1.0 / GN_CNT, None, ALU.mult)
        nc.vector.tensor_scalar(msq[:], gstat_sb[:, 2:4], 1.0 / GN_CNT, None, ALU.mult)
        nc.vector.tensor_mul(mu2[:], mu[:], mu[:])
        nc.vector.tensor_sub(var[:], msq[:], mu2[:])
        nc.vector.tensor_scalar(var[:], var[:], EPS, None, ALU.add)
        nc.scalar.activation(sd[:], var[:], AF.Sqrt, bias=0.0, scale=1.0)
        nc.vector.reciprocal(rsq[:], sd[:])
        for b in range(B):
            nc.vector.tensor_scalar(scale_sb[:, b * 9:(b + 1) * 9], gng_sb[:],
                                    rsq[:, b:b + 1], None, ALU.mult)
            nc.vector.tensor_scalar(tmp9[:], scale_sb[:, b * 9:(b + 1) * 9],
                                    mu[:, b:b + 1], None, ALU.mult)
            nc.vector.tensor_sub(bias_sb[:, b * 9:(b + 1) * 9], gnb_sb[:], tmp9[:])

        stk1.close()

        # =========================================================
        # PHASE 2a: local conv + bn2/swish + gap partials
        # =========================================================
        stk2 = ExitStack()
        p2 = stk2.enter_context(tc.tile_pool(name="p2", bufs=1))
        p2stage = stk2.enter_context(tc.tile_pool(name="p2stage", bufs=2))
        ps_ex = stk2.enter_context(tc.tile_pool(name="ps_ex", bufs=4, space="PSUM"))

        xq_ring = [p2.tile([DIM, LXQ + 2], F32, tag=f"xq{i}", name=f"xq{i}")
                   for i in range(2)]
        for t_ in xq_ring:
            nc.vector.tensor_copy(t_[:, 0:1], ztiny[:, 0:1])
            nc.vector.tensor_copy(t_[:, LXQ + 1:LXQ + 2], ztiny[:, 0:1])

        for f in range(F):
            b = f // T
            xq_t = xq_ring[f % 2]
            nc.sync.dma_start(xq_t[:, 1:LXQ + 1], xq_dram[f])
            yacc = p2stage.tile([DIM, LWV], F32, tag="yacc")
            tmpm = p2stage.tile([DIM, 392], F32, tag="tmpm")
            for ci, (q0, q1) in enumerate(CH_WV):
                n = q1 - q0
                wvr = p2stage.tile([16, 9 * n], F32R, tag="wvraw")
                nc.sync.dma_start(
                    wvr[:],
                    wv_dram[f].rearrange("(g k) l -> g k l", g=16)[:, :, q0:q1])
                wvn = p2stage.tile([16, 9 * n], F32R, tag="wvn")
                for k in range(9):
                    nc.scalar.activation(
                        wvn[:, k * n:(k + 1) * n], wvr[:, k * n:(k + 1) * n],
                        AF.Identity,
                        bias=bias_sb[:, b * 9 + k:b * 9 + k + 1],
                        scale=scale_sb[:, b * 9 + k:b * 9 + k + 1])
                for k in range(9):
                    dh, dw = k // 3, k % 3
                    pe = ps_ex.tile([DIM, n], F32, tag="ex")
                    nc.tensor.matmul(pe[:], E_sb[:], wvn[:, k * n:(k + 1) * n],
                                     start=True, stop=True)
                    off = 1 + q0 + dh * WP + dw - 1
                    if k == 0:
                        nc.vector.tensor_mul(yacc[:, q0:q1], pe[:],
                                             xq_t[:, off:off + n])
                    else:
                        nc.vector.tensor_mul(tmpm[:, :n], pe[:],
                                             xq_t[:, off:off + n])
                        nc.vector.tensor_add(yacc[:, q0:q1], yacc[:, q0:q1],
                                             tmpm[:, :n])
            ysw = p2stage.tile([DIM, LWV], F32, tag="ysw")
            nc.scalar.activation(ysw[:], yacc[:], AF.Silu,
                                 bias=bn2_sb[:, 1:2], scale=bn2_sb[:, 0:1])
            ym = p2stage.tile([DIM, LWV], F32, tag="ym")
            nc.vector.scalar_tensor_tensor(
                ym[:], ysw[:], 1.0, wvmask_sb[:],
                ALU.mult, ALU.mult, accum_out=gapy_sb[:, f:f + 1])
            nc.sync.dma_start(y_dram[f], ym[:])

        # ---- gap AllReduce ----
        nc.vector.tensor_add(gap_sb[:], gap_sb[:], gapy_sb[:])
        nc.sync.dma_start(ar2_in[:, :], gap_sb[:])
        nc.gpsimd.collective_compute(
            "AllReduce", ALU.add, replica_groups=RG,
            ins=[ar2_in[:, :]], outs=[ar2_out[:, :]])
        gap_all = consts.tile([DIM, F], F32)
        nc.sync.dma_start(gap_all[:], ar2_out[:, :])

        # ---- SE block ----
        with tc.tile_pool(name="ps_se", bufs=1, space="PSUM") as ps_se:
            gp = consts.tile([DIM, 20], F32R)
            nc.vector.tensor_copy(gp[:], ztiny[:, 0:20])
            for b in range(B):
                nc.vector.tensor_copy(gp[:, b * 10 + 1:b * 10 + 9],
                                      gap_all[:, b * T:(b + 1) * T])
            p1se = ps_se.tile([64, F], F32, tag="se1")
            for b in range(B):
                for kt in range(K):
                    nc.tensor.matmul(p1se[:, b * T:(b + 1) * T],
                                     se1w_sb[:, kt * 64:(kt + 1) * 64],
                                     gp[:, b * 10 + kt:b * 10 + kt + T],
                                     start=(kt == 0), stop=(kt == 2))
            a1 = consts.tile([64, F], F32)
            nc.scalar.activation(a1[:], p1se[:], AF.Relu,
                                 bias=se1b_sb[:, 0:1], scale=1.0)
            a1p = consts.tile([64, 20], F32R)
            nc.vector.tensor_copy(a1p[:], ztiny[:64, 0:20])
            for b in range(B):
                nc.vector.tensor_copy(a1p[:, b * 10 + 1:b * 10 + 9],
                                      a1[:, b * T:(b + 1) * T])
            pev = ps_se.tile([DIM, F], F32, tag="se2e")
            pod = ps_se.tile([DIM, F], F32, tag="se2o")
            for b in range(B):
                for kt in range(K):
                    nc.tensor.matmul(pev[:, b * T:(b + 1) * T],
                                     se2w_sb[:, kt * 256:kt * 256 + DIM],
                                     a1p[:, b * 10 + kt:b * 10 + kt + T],
                                     start=(kt == 0), stop=(kt == 2))
                    nc.tensor.matmul(pod[:, b * T:(b + 1) * T],
                                     se2w_sb[:, kt * 256 + DIM:kt * 256 + 256],
                                     a1p[:, b * 10 + kt:b * 10 + kt + T],
                                     start=(kt == 0), stop=(kt == 2))
            pev_sb = consts.tile([DIM, F], F32)
            nc.vector.tensor_copy(pev_sb[:], pev[:])
            dse = consts.tile([DIM, F], F32)
            nc.vector.tensor_sub(dse[:], pev_sb[:], pod[:])
            nc.scalar.activation(attn0_sb[:], dse[:], AF.Sigmoid,
                                 bias=se2bd_sb[:, 0:1], scale=1.0)
            if debug:
                nc.sync.dma_start(attn_dbg[:, :], attn0_sb[:])

        stk2.close()

        # =========================================================
        # PHASE 2c: blend + output
        # =========================================================
        with tc.tile_pool(name="p2c", bufs=2) as p2c:
            for f in range(F):
                b, t = f // T, f % T
                yb = p2c.tile([DIM, LWV], F32, tag="yb")
                kb = p2c.tile([DIM, LWV], F32, tag="kb")
                nc.sync.dma_start(yb[:], y_dram[f])
                nc.sync.dma_start(kb[:], kf_dram[f])
                d2 = p2c.tile([DIM, LWV], F32, tag="d2")
                nc.vector.tensor_sub(d2[:], yb[:], kb[:])
                ob = p2c.tile([DIM, LWV], F32, tag="ob")
                nc.vector.scalar_tensor_tensor(
                    ob[:], d2[:], attn0_sb[:, f:f + 1], kb[:], ALU.mult, ALU.add)
                src = ob[:].rearrange("p (r w) -> p r w", r=RWV)[:, :, 1:97]
                nc.sync.dma_start(out_d[b, :, t], src)

    nc.finalize()
    return nc


# =====================================================================
# host-side preparation
# =====================================================================

def _fold_bn(g, b, m, v):
    s = (np.asarray(g, np.float32) / np.sqrt(np.asarray(v, np.float32) + EPS))
    return (s.astype(np.float32),
            (np.asarray(b, np.float32) - np.asarray(m, np.float32) * s).astype(np.float32))


def prep_inputs(inp):
    f32 = np.float32
    x = np.asarray(inp["x"], f32)

    s_ke, b_ke = _fold_bn(inp["ke_g"], inp["ke_b"], inp["ke_m"], inp["ke_v"])
    s_e1, b_e1 = _fold_bn(inp["e1_g"], inp["e1_b"], inp["e1_m"], inp["e1_v"])
    s_c1, b_c1 = _fold_bn(inp["c1_g"], inp["c1_b"], inp["c1_m"], inp["c1_v"])
    s_b2, b_b2 = _fold_bn(inp["bn2_g"], inp["bn2_b"], inp["bn2_m"], inp["bn2_v"])
    s_s1, b_s1 = _fold_bn(inp["se1_g"], inp["se1_b"], inp["se1_m"], inp["se1_v"])

    kew = np.zeros((27, DIM, DIM), f32)
    KW = np.asarray(inp["ke_w"], f32) * s_ke[:, None, None, None, None]
    for kt in range(K):
        for kh in range(K):
            for kw_ in range(K):
                tap = (kt * K + kh) * K + kw_
                for g in range(4):
                    blk = KW[g * 32:(g + 1) * 32, :, kt, kh, kw_]
                    kew[tap, g * 32:(g + 1) * 32, g * 32:(g + 1) * 32] = blk.T
    kew = kew.transpose(1, 0, 2).reshape(DIM, 27 * DIM).copy()

    E1 = np.asarray(inp["e1_w"], f32) * s_e1[:, None, None, None, None]
    e1w = np.zeros((54, DIM, 64), f32)
    for kt in range(K):
        for kh in range(K):
            for kw_ in range(K):
                tap = (kt * K + kh) * K + kw_
                e1w[tap * 2] = E1[:, :DIM, kt, kh, kw_].T
                e1w[tap * 2 + 1] = E1[:, DIM:, kt, kh, kw_].T
    e1w = e1w.transpose(1, 0, 2).reshape(DIM, 54 * 64).copy()

    E2 = np.asarray(inp["e2_w"], f32)
    e2w = np.zeros((27, 64, EMB), f32)
    for kt in range(K):
        for kh in range(K):
            for kw_ in range(K):
                tap = (kt * K + kh) * K + kw_
                e2w[tap] = E2[:, :, kt, kh, kw_].T
    e2w = e2w.transpose(1, 0, 2).reshape(64, 27 * EMB).copy()

    C1 = np.asarray(inp["c1_w"], f32) * s_c1[:, None, None, None, None]
    c1w = np.zeros((27, DIM, DIM), f32)
    for kt in range(K):
        for kh in range(K):
            for kw_ in range(K):
                tap = (kt * K + kh) * K + kw_
                c1w[tap] = C1[:, :, kt, kh, kw_].T
    c1w = c1w.transpose(1, 0, 2).reshape(DIM, 27 * DIM).copy()

    S1 = np.asarray(inp["se1_w"], f32)[:, :, :, 1, 1] * s_s1[:, None, None]
    se1w = np.zeros((DIM, 3 * 64), f32)
    for kt in range(K):
        se1w[:, kt * 64:(kt + 1) * 64] = (S1[:, :, kt] / (H * W)).T
    se1b = (np.asarray(inp["se1_bias"], f32) * s_s1 + b_s1).reshape(64, 1)

    S2 = np.asarray(inp["se2_w"], f32)[:, :, :, 1, 1]
    se2w = np.zeros((64, 3 * 256), f32)
    for kt in range(K):
        se2w[:, kt * 256:kt * 256 + DIM] = S2[0::2, :, kt].T
        se2w[:, kt * 256 + DIM:kt * 256 + 256] = S2[1::2, :, kt].T
    se2b = np.asarray(inp["se2_bias"], f32)
    se2bd = (se2b[0::2] - se2b[1::2]).reshape(DIM, 1)

    Emat = np.zeros((16, DIM), f32)
    for c in range(DIM):
        Emat[c // SHARE, c] = 1.0
    Gmat = np.zeros((DIM, 16), f32)
    for ch in range(DIM):
        Gmat[ch, ch // 9] = 1.0
    Ghmat = np.zeros((16, 16), f32)
    for i_ in range(16):
        Ghmat[i_, (DIM + i_) // 9] = 1.0

    shared = dict(
        kew=kew, keb=b_ke.reshape(DIM, 1),
        e1w=e1w, e1b=b_e1.reshape(64, 1),
        e2w=e2w, e2b=np.asarray(inp["e2_bias"], f32).reshape(EMB, 1),
        c1w=c1w, c1b=b_c1.reshape(DIM, 1),
        bn2s=np.stack([s_b2, b_b2], axis=1).astype(f32),
        se1w=se1w, se1b=se1b, se2w=se2w, se2bd=se2bd,
        gng=np.asarray(inp["gn_g"], f32).reshape(16, 9),
        gnb=np.asarray(inp["gn_b"], f32).reshape(16, 9),
        Emat=Emat, Gmat=Gmat, Ghmat=Ghmat,
    )

    def mk_mask(parts, rows, base):
        m = np.zeros((parts, rows, WP), f32)
        for i in range(rows):
            if 0 <= base + i < H:
                m[:, i, 1:97] = 1.0
        return m.reshape(parts, rows * WP)

    per_core = []
    for c in range(NCORES):
        own0 = c * ROWN
        xc = np.zeros((F, DIM, RX, WP), f32)
        r0 = own0 - 3
        lo, hi = max(0, r0), min(H, r0 + RX)
        for b in range(B):
            xc[b * T:(b + 1) * T, :, lo - r0:hi - r0, 1:97] = \
                x[b].transpose(1, 0, 2, 3)[:, :, lo:hi, :]
        d = dict(shared)
        d["xin"] = xc.reshape(F, DIM, LX)
        d["kmask"] = mk_mask(DIM, RKF, own0 - 2)
        d["e1mask"] = mk_mask(64, RE1, own0 - 1)
        d["wvmask"] = mk_mask(DIM, RWV, own0)
        d["xqmask"] = mk_mask(DIM, RXQ, own0 - 1)
        per_core.append(d)
    return per_core


# =====================================================================
# resident runner
# =====================================================================
class _Runner:
    def __init__(self, nc, n_cores):
        import jax
        from jax.sharding import Mesh, PartitionSpec, NamedSharding
        from jax.experimental.shard_map import shard_map
        from concourse.bass2jax import (_bass_exec_p, install_neuronx_cc_hook,
                                        partition_id_tensor)
        install_neuronx_cc_hook()
        self.jax = jax
        self.nc = nc
        self.n_cores = n_cores

        in_names, out_names, out_avals, zero_outs = [], [], [], []
        pid_name = nc.partition_id_tensor.name if nc.partition_id_tensor else None
        for alloc in nc.m.functions[0].allocations:
            if not isinstance(alloc, mybir.MemoryLocationSet):
                continue
            name = alloc.memorylocations[0].name
            if alloc.kind == "ExternalInput":
                if name != pid_name:
                    in_names.append(name)
            elif alloc.kind == "ExternalOutput":
                out_names.append(name)
                out_avals.append(jax.core.ShapedArray(
                    tuple(alloc.tensor_shape), mybir.dt.np(alloc.dtype)))
                zero_outs.append(np.zeros(tuple(alloc.tensor_shape),
                                          mybir.dt.np(alloc.dtype)))
        self.in_names, self.out_names = in_names, out_names
        self.out_avals, self.zero_outs = out_avals, zero_outs
        n_params = len(in_names)
        all_in_names = list(in_names) + list(out_names)
        if pid_name is not None:
            all_in_names.append(pid_name)
        has_pid = pid_name is not None

        def _body(*args):
            operands = list(args)
            if has_pid:
                operands.append(partition_id_tensor())
            return tuple(_bass_exec_p.bind(
                *operands,
                out_avals=tuple(out_avals),
                in_names=tuple(all_in_names),
                out_names=tuple(out_names),
                lowering_input_output_aliases=(),
                sim_require_finite=True,
                sim_require_nnan=True,
                nc=nc,
            ))

        devices = jax.devices()[:n_cores]
        self.mesh = Mesh(np.asarray(devices), ("core",))
        self.sharding = NamedSharding(self.mesh, PartitionSpec("core"))
        in_specs = (PartitionSpec("core"),) * (n_params + len(out_names))
        out_specs = (PartitionSpec("core"),) * len(out_names)
        self.fn = jax.jit(
            shard_map(_body, mesh=self.mesh, in_specs=in_specs,
                      out_specs=out_specs, check_rep=False),
            keep_unused=True)
        self._zero_dev = None

    def put_inputs(self, in_maps):
        jax = self.jax
        concat = [np.concatenate([np.asarray(in_maps[c][n])
                                  for c in range(self.n_cores)], axis=0)
                  for n in self.in_names]
        if self._zero_dev is None:
            zeros = [np.concatenate([z] * self.n_cores, axis=0)
                     for z in self.zero_outs]
            self._zero_dev = [jax.device_put(z, self.sharding) for z in zeros]
        self._dev_in = [jax.device_put(a, self.sharding) for a in concat]
        jax.block_until_ready(self._dev_in)

    def run(self):
        return self.fn(*self._dev_in, *self._zero_dev)

    def run_np(self):
        jax = self.jax
        out = jax.block_until_ready(self.run())
        res = []
        for c in range(self.n_cores):
            d = {}
            for i, name in enumerate(self.out_names):
                full = np.asarray(out[i])
                d[name] = full.reshape(self.n_cores, *self.out_avals[i].shape)[c]
            res.append(d)
        return res


_CACHE = {}


def _get_runner(debug=False):
    key = ("runner", debug)
    if key not in _CACHE:
        nc = build_nc(debug=debug)
        _CACHE[key] = _Runner(nc, NCORES)
    return _CACHE[key]


def kernel(**inputs) -> np.ndarray:
    per_core = prep_inputs(inputs)
    r = _get_runner(debug=False)
    r.put_inputs(per_core)
    res = r.run_np()
    out = np.concatenate([res[c]["out"] for c in range(NCORES)], axis=3)
    return out.astype(np.float32)



# revision 26
# speedup vs baseline: 1.2964x; 1.2964x over previous
"""Trainium2 Bass kernel for nn_CotLayer (CoT attention layer).

Strategy: H-dimension sharding across 8 NeuronCores (12 of 96 rows each, all
B*T frames per core, halo rows included host-side).  All 3x3x3 convs are
implicit GEMM: channels on partitions, padded-width (98) pixel lines on the
free dim, 27 shifted matmuls accumulating in PSUM (fp32r = full PE rate).
GroupNorm statistics and the split-attention gap are AllReduced across cores.

Self-contained: only numpy/jax/concourse imports.
"""
from contextlib import ExitStack

import numpy as np

import concourse.bacc as bacc
import concourse.tile as tile
from concourse import mybir

F32 = mybir.dt.float32
F32R = mybir.dt.float32r
BF16 = mybir.dt.bfloat16
AF = mybir.ActivationFunctionType
ALU = mybir.AluOpType

# problem constants
B, DIM, T, H, W = 2, 128, 8, 96, 96
K, SHARE, RADIX = 3, 8, 2
EMB = 144
EPS = 1e-5
NCORES = 8
F = B * T               # 16 frames
ROWN = H // NCORES      # 12 owned rows per core
WP = W + 2              # padded width

# slab geometry (rows, line length)
RX, RKF, RE1, RWV, RXQ = 18, 16, 14, 12, 14
LX, LKF, LE1, LWV, LXQ = RX * WP, RKF * WP, RE1 * WP, RWV * WP, RXQ * WP

GN_CNT = float((EMB // 16) * T * H * W)   # 9*8*96*96 per (b, group)


def _chunks(L, n):
    # fp32r matmul requires even moving-operand counts -> even chunk sizes
    assert L % 2 == 0
    base2 = (L // n) // 2 * 2
    rem = L - base2 * n
    out, s = [], 0
    for i in range(n):
        e = s + base2 + (2 if i < rem // 2 else 0)
        out.append((s, e))
        s = e
    assert s == L
    return out


CH_KF = _chunks(LKF, 4)
CH_E1 = _chunks(LE1, 3)
CH_WV = _chunks(LWV, 3)
CH_XQ = _chunks(LXQ, 3)


def _same_batch(f, g):
    return 0 <= g < F and g // T == f // T


def build_nc(debug=False, ncores=NCORES):
    nc = bacc.Bacc()

    # ---------------- I/O ----------------
    xin = nc.dram_tensor("xin", [F, DIM, LX], F32R, kind="ExternalInput")
    kew = nc.dram_tensor("kew", [DIM, 27 * DIM], F32R, kind="ExternalInput")
    keb = nc.dram_tensor("keb", [DIM, 1], F32, kind="ExternalInput")
    e1w = nc.dram_tensor("e1w", [DIM, 54 * 64], F32R, kind="ExternalInput")
    e1b = nc.dram_tensor("e1b", [64, 1], F32, kind="ExternalInput")
    e2wA = nc.dram_tensor("e2wA", [DIM, 9 * EMB], F32R, kind="ExternalInput")
    e2wA1 = nc.dram_tensor("e2wA1", [64, 9 * EMB], F32R, kind="ExternalInput")
    e2wB = nc.dram_tensor("e2wB", [DIM, 3 * EMB], F32R, kind="ExternalInput")
    e2wC = nc.dram_tensor("e2wC", [64, 3 * EMB], F32R, kind="ExternalInput")
    e2b = nc.dram_tensor("e2b", [EMB, 1], F32, kind="ExternalInput")
    c1w = nc.dram_tensor("c1w", [DIM, 27 * DIM], F32R, kind="ExternalInput")
    c1b = nc.dram_tensor("c1b", [DIM, 1], F32, kind="ExternalInput")
    bn2s = nc.dram_tensor("bn2s", [DIM, 2], F32, kind="ExternalInput")
    se1w_d = nc.dram_tensor("se1w", [DIM, 3 * 64], F32R, kind="ExternalInput")
    se1b_d = nc.dram_tensor("se1b", [64, 1], F32, kind="ExternalInput")
    se2w_d = nc.dram_tensor("se2w", [64, 3 * 256], F32R, kind="ExternalInput")
    se2bd_d = nc.dram_tensor("se2bd", [DIM, 1], F32, kind="ExternalInput")
    gng_d = nc.dram_tensor("gng", [16, 9], F32, kind="ExternalInput")
    gnb_d = nc.dram_tensor("gnb", [16, 9], F32, kind="ExternalInput")
    E_d = nc.dram_tensor("Emat", [16, DIM], BF16, kind="ExternalInput")
    Ef_d = nc.dram_tensor("Ematf", [16, DIM], F32R, kind="ExternalInput")
    G_d = nc.dram_tensor("Gmat", [DIM, 16], F32, kind="ExternalInput")
    Gh_d = nc.dram_tensor("Ghmat", [16, 16], F32, kind="ExternalInput")
    kmask_d = nc.dram_tensor("kmask", [DIM, LKF], BF16, kind="ExternalInput")
    e1mask_d = nc.dram_tensor("e1mask", [64, LE1], BF16, kind="ExternalInput")
    wvmask_d = nc.dram_tensor("wvmask", [DIM, LWV], F32, kind="ExternalInput")
    xqmask_d = nc.dram_tensor("xqmask", [DIM, LXQ], BF16, kind="ExternalInput")

    out_d = nc.dram_tensor("out", [B, DIM, T, ROWN, W], F32, kind="ExternalOutput")

    dbg = "ExternalOutput" if debug else "Internal"
    wv_dram = nc.dram_tensor("wv_s", [F, EMB, LWV], BF16, kind=dbg)
    xq_dram = nc.dram_tensor("xq_s", [F, DIM, LXQ], BF16, kind=dbg)
    kf_dram = nc.dram_tensor("kf_s", [F, DIM, LWV], BF16, kind=dbg)
    y_dram = nc.dram_tensor("y_s", [F, DIM, LWV], BF16, kind=dbg)
    if debug:
        attn_dbg = nc.dram_tensor("attn_dbg", [DIM, 16], F32, kind="ExternalOutput")
        gstat_dbg = nc.dram_tensor("gstat_dbg", [16, 4], F32, kind="ExternalOutput")

    shr = {} if ncores == 1 else {"addr_space": "Shared"}
    ar1_in = nc.dram_tensor("ar1_in", [16, 4], F32)
    ar1_out = nc.dram_tensor("ar1_out", [16, 4], F32, **shr)
    ar2_in = nc.dram_tensor("ar2_in", [DIM, F], F32)
    ar2_out = nc.dram_tensor("ar2_out", [DIM, F], F32, **shr)
    RG = [list(range(ncores))]

    with tile.TileContext(nc) as tc, ExitStack() as stk:
        consts = stk.enter_context(tc.tile_pool(name="consts", bufs=1))

        def load_const(dram, p, l, dt=F32):
            t = consts.tile([p, l], dt, name=dram.name + "_sb")
            nc.sync.dma_start(t[:], dram[:, :])
            return t

        kew_sb = load_const(kew, DIM, 27 * DIM, F32R)
        e1w_sb = load_const(e1w, DIM, 54 * 64, F32R)
        e2wA_sb = load_const(e2wA, DIM, 9 * EMB, F32R)
        e2wA1_sb = load_const(e2wA1, 64, 9 * EMB, F32R)
        e2wB_sb = load_const(e2wB, DIM, 3 * EMB, F32R)
        e2wC_sb = load_const(e2wC, 64, 3 * EMB, F32R)
        c1w_sb = load_const(c1w, DIM, 27 * DIM, F32R)
        se1w_sb = load_const(se1w_d, DIM, 3 * 64, F32R)
        se2w_sb = load_const(se2w_d, 64, 3 * 256, F32R)
        E_sb = load_const(E_d, 16, DIM, BF16)
        Ef_sb = load_const(Ef_d, 16, DIM, F32R)
        G_sb = load_const(G_d, DIM, 16)
        Gh_sb = load_const(Gh_d, 16, 16)
        keb_sb = load_const(keb, DIM, 1)
        e1b_sb = load_const(e1b, 64, 1)
        c1b_sb = load_const(c1b, DIM, 1)
        bn2_sb = load_const(bn2s, DIM, 2)
        se1b_sb = load_const(se1b_d, 64, 1)
        se2bd_sb = load_const(se2bd_d, DIM, 1)
        gng_sb = load_const(gng_d, 16, 9)
        gnb_sb = load_const(gnb_d, 16, 9)
        wvmask_sb = load_const(wvmask_d, DIM, LWV)
        e2b_lo_sb = consts.tile([DIM, 1], F32)
        nc.sync.dma_start(e2b_lo_sb[:], e2b[0:DIM, :])
        e2b_hi_sb = consts.tile([16, 1], F32)
        nc.sync.dma_start(e2b_hi_sb[:], e2b[DIM:EMB, :])

        ztiny = consts.tile([DIM, 24], F32)
        nc.vector.memset(ztiny[:], 0.0)

        # stats / gap accumulators
        ssum_lo = consts.tile([DIM, 3 * F], F32)
        ssq_lo = consts.tile([DIM, 3 * F], F32)
        ssum_hi = consts.tile([16, 3 * F], F32)
        ssq_hi = consts.tile([16, 3 * F], F32)
        gap_sb = consts.tile([DIM, F], F32)
        gapy_sb = consts.tile([DIM, F], F32)
        gstat_sb = consts.tile([16, 4], F32)
        scale_sb = consts.tile([16, 2 * 9], F32)
        bias_sb = consts.tile([16, 2 * 9], F32)
        attn0_sb = consts.tile([DIM, F], F32)
        sred_lo = consts.tile([DIM, 4], F32)
        sred_hi = consts.tile([16, 4], F32)

        # =========================================================
        # PHASE 1
        # =========================================================
        stk1 = ExitStack()
        p1 = stk1.enter_context(tc.tile_pool(name="p1", bufs=1))
        p1stage = stk1.enter_context(tc.tile_pool(name="p1stage", bufs=2))
        stk1ps = ExitStack()
        ps_ke = stk1ps.enter_context(tc.tile_pool(name="ps_ke", bufs=2, space="PSUM"))
        ps_e1 = stk1ps.enter_context(tc.tile_pool(name="ps_e1", bufs=2, space="PSUM"))
        ps_e2l = stk1ps.enter_context(tc.tile_pool(name="ps_e2l", bufs=2, space="PSUM"))
        ps_e2h = stk1ps.enter_context(tc.tile_pool(name="ps_e2h", bufs=2, space="PSUM"))

        kmask_sb = p1.tile([DIM, LKF], BF16)
        nc.sync.dma_start(kmask_sb[:], kmask_d[:, :])
        e1mask_sb = p1.tile([64, LE1], BF16)
        nc.sync.dma_start(e1mask_sb[:], e1mask_d[:, :])
        xqmask_sb = p1.tile([DIM, LXQ], BF16)
        nc.sync.dma_start(xqmask_sb[:], xqmask_d[:, :])

        zslab = p1.tile([DIM, LX + 2], F32R)
        zsf = p1.tile([DIM, LX + 2], F32)
        nc.vector.memset(zsf[:], 0.0)
        nc.vector.tensor_copy(zslab[:], zsf[:])

        x_ring = [p1.tile([DIM, LX + 2], F32R, tag=f"xr{i}", name=f"xr{i}")
                  for i in range(4)]
        kf_ring = [p1.tile([DIM, LKF + 2], F32R, tag=f"kfr{i}", name=f"kfr{i}")
                   for i in range(3)]
        # e2 frame-stacked moving tiles:
        #  pair_ring[f%3]: rows 0:64 = e1[f-1], rows 64:128 = e1[f]
        #  s2x_ring[f%2]:  rows 0:64 = e1[f+1], rows 64:128 = e1[f+1] shifted +WP
        pair_ring = [p1.tile([DIM, LE1 + 2], F32R, tag=f"pr{i}", name=f"pr{i}")
                     for i in range(3)]
        s2x_ring = [p1.tile([DIM, LE1 + 2], F32R, tag=f"s2x{i}", name=f"s2x{i}")
                    for i in range(2)]
        edge_t = p1.tile([64, LE1 + 2], F32R, name="edge_t")
        for t_ in pair_ring + s2x_ring + [edge_t]:
            pp = t_.shape[0]
            nc.vector.tensor_copy(t_[:], zsf[:pp, 0:LE1 + 2])
        for t_ in x_ring + kf_ring:
            pp = t_.shape[0]
            L = t_.shape[1]
            nc.vector.tensor_copy(t_[:, 0:1], zslab[:pp, 0:1])
            nc.vector.tensor_copy(t_[:, L - 1:L], zslab[:pp, 0:1])

        def load_x(f):
            nc.sync.dma_start(x_ring[f % 4][:, 1:LX + 1], xin[f])

        def xsrc(f, d):
            return x_ring[(f + d) % 4] if _same_batch(f, f + d) else zslab

        def kfsrc(f, d):
            return kf_ring[(f + d) % 3] if _same_batch(f, f + d) else zslab

        # ---------------- PHASE 1a ----------------
        load_x(0)
        for i in range(F + 2):
            if i + 1 < F:
                load_x(i + 1)
            # A: kf[i]
            if i < F:
                kf_t = kf_ring[i % 3]
                for (q0, q1) in CH_KF:
                    n = q1 - q0
                    p = ps_ke.tile([DIM, n], F32, tag="ke")
                    for kt in range(K):
                        src = xsrc(i, kt - 1)
                        for kh in range(K):
                            for kw in range(K):
                                tap = (kt * K + kh) * K + kw
                                off = 1 + q0 + kh * WP + kw - 1
                                nc.tensor.matmul(
                                    p[:], kew_sb[:, tap * DIM:(tap + 1) * DIM],
                                    src[:, off:off + n],
                                    start=(tap == 0), stop=(tap == 26))
                    nc.scalar.activation(kf_t[:, 1 + q0:1 + q1], p[:], AF.Relu,
                                         bias=keb_sb[:, 0:1], scale=1.0)
                nc.vector.tensor_mul(kf_t[:, 1:LKF + 1], kf_t[:, 1:LKF + 1],
                                     kmask_sb[:])
                kstore = p1stage.tile([DIM, LWV], BF16, tag="kfstore")
                own = kf_t[:, 1:LKF + 1].rearrange("p (r w) -> p r w", r=RKF)[:, 2:14, :]
                nc.scalar.activation(
                    kstore[:].rearrange("p (r w) -> p r w", r=RWV), own,
                    AF.Identity, bias=0.0, scale=1.0,
                    accum_out=gap_sb[:, i:i + 1])
                nc.sync.dma_start(kf_dram[i], kstore[:])
            # B: e1out[i-1] (before x[i+1] prefetch overwrites x[i-2])
            tb = i - 1
            if 0 <= tb < F:
                e1_t = p1stage.tile([64, LE1], F32R, tag="e1st")
                for (q0, q1) in CH_E1:
                    n = q1 - q0
                    p = ps_e1.tile([64, n], F32, tag="e1")
                    for kt in range(K):
                        sx = xsrc(tb, kt - 1)
                        sk = kfsrc(tb, kt - 1)
                        for kh in range(K):
                            for kw in range(K):
                                tap = (kt * K + kh) * K + kw
                                offx = 1 + q0 + (kh + 1) * WP + kw - 1
                                offk = 1 + q0 + kh * WP + kw - 1
                                c0 = (tap * 2) * 64
                                nc.tensor.matmul(
                                    p[:], e1w_sb[:, c0:c0 + 64],
                                    sx[:, offx:offx + n],
                                    start=(tap == 0), stop=False)
                                nc.tensor.matmul(
                                    p[:], e1w_sb[:, c0 + 64:c0 + 128],
                                    sk[:, offk:offk + n],
                                    start=False, stop=(tap == 26))
                    nc.scalar.activation(e1_t[:, q0:q1], p[:], AF.Relu,
                                         bias=e1b_sb[:, 0:1], scale=1.0)
                nc.vector.tensor_mul(e1_t[:], e1_t[:], e1mask_sb[:])
                # scatter into the frame-stacked e2 moving tiles
                nc.sync.dma_start(pair_ring[tb % 3][64:128, 1:1 + LE1], e1_t[:])
                if tb % T != T - 1:
                    nc.sync.dma_start(pair_ring[(tb + 1) % 3][0:64, 1:1 + LE1],
                                      e1_t[:])
                if tb % T != 0:
                    s2w = s2x_ring[(tb - 1) % 2]
                    nc.sync.dma_start(s2w[0:64, 1:1 + LE1], e1_t[:])
                    nc.sync.dma_start(s2w[64:128, 0:LE1 - WP + 1],
                                      e1_t[:, WP - 1:LE1])
                else:
                    nc.sync.dma_start(edge_t[:, 1:1 + LE1], e1_t[:])
            # C: wv[i-2]  (e2 conv, frame-stacked: <=15 streams/chunk/half)
            f2 = i - 2
            if 0 <= f2 < F:
                t_in_b = f2 % T
                s2 = s2x_ring[f2 % 2]
                n_mm = 9 + (6 if t_in_b != T - 1 else 0)
                wlo = p1stage.tile([DIM, LWV], BF16, tag="wvlo")
                whi = p1stage.tile([16, LWV], BF16, tag="wvhi")
                sqj = p1stage.tile([DIM, 392], F32, tag="sqjunk")
                for ci, (q0, q1) in enumerate(CH_WV):
                    n = q1 - q0
                    plo = ps_e2l.tile([DIM, n], F32, tag="e2l")
                    phi = ps_e2h.tile([16, n], F32, tag="e2h")
                    mm = 0

                    def emit(w_sb, c0, mv_sl, nmm=n_mm, plo=plo, phi=phi):
                        nonlocal mm
                        nc.tensor.matmul(plo[:], w_sb[:, c0:c0 + DIM], mv_sl,
                                         start=(mm == 0), stop=(mm == nmm - 1))
                        nc.tensor.matmul(phi[:], w_sb[:, c0 + DIM:c0 + EMB],
                                         mv_sl,
                                         start=(mm == 0), stop=(mm == nmm - 1))
                        mm += 1

                    for kh in range(K):
                        for kw in range(K):
                            c0 = (kh * K + kw) * EMB
                            off = 1 + q0 + kh * WP + kw - 1
                            if t_in_b == 0:
                                emit(e2wA1_sb, c0, edge_t[:, off:off + n])
                            else:
                                emit(e2wA_sb, c0,
                                     pair_ring[f2 % 3][:, off:off + n])
                    if t_in_b != T - 1:
                        for kw in range(K):
                            off = 1 + q0 + kw - 1
                            emit(e2wB_sb, kw * EMB, s2[:, off:off + n])
                        for kw in range(K):
                            off = 1 + q0 + 2 * WP + kw - 1
                            emit(e2wC_sb, kw * EMB, s2[0:64, off:off + n])
                    assert mm == n_mm
                    col = f2 * 3 + ci
                    nc.vector.scalar_tensor_tensor(
                        wlo[:, q0:q1], plo[:], e2b_lo_sb[:, 0:1],
                        wvmask_sb[:, q0:q1],
                        ALU.add, ALU.mult, accum_out=ssum_lo[:, col:col + 1])
                    nc.vector.scalar_tensor_tensor(
                        whi[:, q0:q1], phi[:], e2b_hi_sb[:, 0:1],
                        wvmask_sb[:16, q0:q1],
                        ALU.add, ALU.mult, accum_out=ssum_hi[:, col:col + 1])
                    nc.scalar.activation(sqj[:, :n], wlo[:, q0:q1], AF.Square,
                                         bias=0.0, scale=1.0,
                                         accum_out=ssq_lo[:, col:col + 1])
                    nc.scalar.activation(sqj[:16, :n], whi[:, q0:q1], AF.Square,
                                         bias=0.0, scale=1.0,
                                         accum_out=ssq_hi[:, col:col + 1])
                nc.sync.dma_start(wv_dram[f2, 0:DIM], wlo[:])
                nc.sync.dma_start(wv_dram[f2, DIM:EMB], whi[:])

        # ---- GN stats reduce + AllReduce ----
        stk1ps.close()
        half = 3 * T
        nc.vector.reduce_sum(sred_lo[:, 0:1], ssum_lo[:, 0:half], axis=mybir.AxisListType.X)
        nc.vector.reduce_sum(sred_lo[:, 1:2], ssum_lo[:, half:], axis=mybir.AxisListType.X)
        nc.vector.reduce_sum(sred_lo[:, 2:3], ssq_lo[:, 0:half], axis=mybir.AxisListType.X)
        nc.vector.reduce_sum(sred_lo[:, 3:4], ssq_lo[:, half:], axis=mybir.AxisListType.X)
        nc.vector.reduce_sum(sred_hi[:, 0:1], ssum_hi[:, 0:half], axis=mybir.AxisListType.X)
        nc.vector.reduce_sum(sred_hi[:, 1:2], ssum_hi[:, half:], axis=mybir.AxisListType.X)
        nc.vector.reduce_sum(sred_hi[:, 2:3], ssq_hi[:, 0:half], axis=mybir.AxisListType.X)
        nc.vector.reduce_sum(sred_hi[:, 3:4], ssq_hi[:, half:], axis=mybir.AxisListType.X)
        with tc.tile_pool(name="ps_st", bufs=1, space="PSUM") as ps_st:
            pst = ps_st.tile([16, 4], F32, tag="gstat")
            nc.tensor.matmul(pst[:], G_sb[:], sred_lo[:], start=True, stop=False)
            nc.tensor.matmul(pst[:], Gh_sb[:], sred_hi[:], start=False, stop=True)
            gloc = consts.tile([16, 4], F32)
            nc.vector.tensor_copy(gloc[:], pst[:])
        nc.sync.dma_start(ar1_in[:, :], gloc[:])
        if ncores == 1:   # sim-profiling build: skip collectives
            nc.sync.dma_start(ar1_out[:, :], ar1_in[:, :])
        else:
            nc.gpsimd.collective_compute(
                "AllReduce", ALU.add, replica_groups=RG,
                ins=[ar1_in[:, :]], outs=[ar1_out[:, :]])
        nc.sync.dma_start(gstat_sb[:], ar1_out[:, :])
        if debug:
            nc.sync.dma_start(gstat_dbg[:, :], gstat_sb[:])

        # ---------------- PHASE 1b: c1 (overlaps AR1) ----------------
        with tc.tile_pool(name="ps_c1", bufs=3, space="PSUM") as ps_c1:
            load_x(0)
            for f in range(F):
                if f + 1 < F:
                    load_x(f + 1)
                xst = p1stage.tile([DIM, LXQ], BF16, tag="xqstage")
                for (q0, q1) in CH_XQ:
                    n = q1 - q0
                    p = ps_c1.tile([DIM, n], F32, tag="c1")
                    for kt in range(K):
                        src = xsrc(f, kt - 1)
                        for kh in range(K):
                            for kw in range(K):
                                tap = (kt * K + kh) * K + kw
                                off = 1 + q0 + (kh + 1) * WP + kw - 1
                                nc.tensor.matmul(
                                    p[:], c1w_sb[:, tap * DIM:(tap + 1) * DIM],
                                    src[:, off:off + n],
                                    start=(tap == 0), stop=(tap == 26))
                    nc.vector.scalar_tensor_tensor(
                        xst[:, q0:q1], p[:], c1b_sb[:, 0:1], xqmask_sb[:, q0:q1],
                        ALU.add, ALU.mult)
                nc.sync.dma_start(xq_dram[f], xst[:])

        # ---- GN scale/bias ----
        mu = consts.tile([16, 2], F32)
        msq = consts.tile([16, 2], F32)
        mu2 = consts.tile([16, 2], F32)
        var = consts.tile([16, 2], F32)
        sd = consts.tile([16, 2], F32)
        rsq = consts.tile([16, 2], F32)
        tmp9 = consts.tile([16, 9], F32)
        nc.vector.tensor_scalar(mu[:], gstat_sb[:, 0:2], 1.0 / GN_CNT, None, ALU.mult)
        nc.vector.tensor_scalar(msq[:], gstat_sb[:, 2:4], 1.0 / GN_CNT, None, ALU.mult)
        nc.vector.tensor_mul(mu2[:], mu[:], mu[:])
        nc.vector.tensor_sub(var[:], msq[:], mu2[:])
        nc.vector.tensor_scalar(var[:], var[:], EPS, None, ALU.add)
        nc.scalar.activation(sd[:], var[:], AF.Sqrt, bias=0.0, scale=1.0)
        nc.vector.reciprocal(rsq[:], sd[:])
        for b in range(B):
            nc.vector.tensor_scalar(scale_sb[:, b * 9:(b + 1) * 9], gng_sb[:],
                                    rsq[:, b:b + 1], None, ALU.mult)
            nc.vector.tensor_scalar(tmp9[:], scale_sb[:, b * 9:(b + 1) * 9],
                                    mu[:, b:b + 1], None, ALU.mult)
            nc.vector.tensor_sub(bias_sb[:, b * 9:(b + 1) * 9], gnb_sb[:], tmp9[:])

        stk1.close()

        # =========================================================
        # PHASE 2a: local conv + bn2/swish + gap partials
        # =========================================================
        stk2 = ExitStack()
        p2 = stk2.enter_context(tc.tile_pool(name="p2", bufs=1))
        p2stage = stk2.enter_context(tc.tile_pool(name="p2stage", bufs=2))
        ps_ex = stk2.enter_context(tc.tile_pool(name="ps_ex", bufs=4, space="PSUM"))

        # GN affine folded into the broadcast: ek = E * scale(b,k) (stationary),
        # tbias[c, b*9+k] = bias(g(c), b, k) applied via the stt scalar.
        ek_sb = p2.tile([16, 2 * 9 * DIM], BF16, name="ek")
        for b in range(B):
            for k in range(9):
                j = b * 9 + k
                nc.vector.tensor_scalar(ek_sb[:, j * DIM:(j + 1) * DIM], E_sb[:],
                                        scale_sb[:, j:j + 1], None, ALU.mult)
        biasr = p2.tile([16, 2 * 9], F32R)
        nc.vector.tensor_copy(biasr[:], bias_sb[:])
        tbias = p2.tile([DIM, 2 * 9], F32)
        with tc.tile_pool(name="ps_tb", bufs=1, space="PSUM") as ps_tb:
            ptb = ps_tb.tile([DIM, 2 * 9], F32, tag="tb")
            nc.tensor.matmul(ptb[:], Ef_sb[:], biasr[:], start=True, stop=True)
            nc.vector.tensor_copy(tbias[:], ptb[:])

        xq_ring = [p2.tile([DIM, LXQ + 2], BF16, tag=f"xq{i}", name=f"xq{i}")
                   for i in range(2)]
        for t_ in xq_ring:
            nc.vector.memset(t_[:, 0:1], 0.0)
            nc.vector.memset(t_[:, LXQ + 1:LXQ + 2], 0.0)

        for f in range(F):
            b = f // T
            xq_t = xq_ring[f % 2]
            nc.sync.dma_start(xq_t[:, 1:LXQ + 1], xq_dram[f])
            wvr = p2stage.tile([16, 9 * LWV], BF16, tag="wvraw")
            nc.sync.dma_start(
                wvr[:].rearrange("p (k l) -> p k l", k=9),
                wv_dram[f].rearrange("(g k) l -> g k l", g=16))
            yacc = p2stage.tile([DIM, LWV], BF16, tag="yacc")
            tmpm = p2stage.tile([DIM, 392], BF16, tag="tmpm")
            peb = p2stage.tile([DIM, 9 * 392], BF16, tag="peb")
            for ci, (q0, q1) in enumerate(CH_WV):
                n = q1 - q0
                for k in range(9):
                    j = b * 9 + k
                    pe = ps_ex.tile([DIM, n], F32, tag="ex")
                    nc.tensor.matmul(pe[:], ek_sb[:, j * DIM:(j + 1) * DIM],
                                     wvr[:, k * LWV + q0:k * LWV + q1],
                                     start=True, stop=True)
                    nc.scalar.activation(peb[:, k * n:(k + 1) * n], pe[:],
                                         AF.Identity, bias=0.0, scale=1.0)
                for k in range(9):
                    dh, dw = k // 3, k % 3
                    j = b * 9 + k
                    off = 1 + q0 + dh * WP + dw - 1
                    if k == 0:
                        nc.vector.scalar_tensor_tensor(
                            yacc[:, q0:q1], peb[:, 0:n], tbias[:, j:j + 1],
                            xq_t[:, off:off + n], ALU.add, ALU.mult)
                    else:
                        nc.vector.scalar_tensor_tensor(
                            tmpm[:, :n], peb[:, k * n:(k + 1) * n],
                            tbias[:, j:j + 1],
                            xq_t[:, off:off + n], ALU.add, ALU.mult)
                        nc.vector.tensor_add(yacc[:, q0:q1], yacc[:, q0:q1],
                                             tmpm[:, :n])
            ysw = p2stage.tile([DIM, LWV], BF16, tag="ysw")
            nc.scalar.activation(ysw[:], yacc[:],
                                 AF.Tanh if ncores == 1 else AF.Silu,
                                 bias=bn2_sb[:, 1:2], scale=bn2_sb[:, 0:1])
            ym = p2stage.tile([DIM, LWV], BF16, tag="ym")
            nc.vector.scalar_tensor_tensor(
                ym[:], ysw[:], 1.0, wvmask_sb[:],
                ALU.mult, ALU.mult, accum_out=gapy_sb[:, f:f + 1])
            nc.sync.dma_start(y_dram[f], ym[:])

        # ---- gap AllReduce ----
        nc.vector.tensor_add(gap_sb[:], gap_sb[:], gapy_sb[:])
        nc.sync.dma_start(ar2_in[:, :], gap_sb[:])
        if ncores == 1:   # sim-profiling build: skip collectives
            nc.sync.dma_start(ar2_out[:, :], ar2_in[:, :])
        else:
            nc.gpsimd.collective_compute(
                "AllReduce", ALU.add, replica_groups=RG,
                ins=[ar2_in[:, :]], outs=[ar2_out[:, :]])
        gap_all = consts.tile([DIM, F], F32)
        nc.sync.dma_start(gap_all[:], ar2_out[:, :])

        # ---- SE block ----
        with tc.tile_pool(name="ps_se", bufs=1, space="PSUM") as ps_se:
            gp = consts.tile([DIM, 20], F32R)
            nc.vector.tensor_copy(gp[:], ztiny[:, 0:20])
            for b in range(B):
                nc.vector.tensor_copy(gp[:, b * 10 + 1:b * 10 + 9],
                                      gap_all[:, b * T:(b + 1) * T])
            p1se = ps_se.tile([64, F], F32, tag="se1")
            for b in range(B):
                for kt in range(K):
                    nc.tensor.matmul(p1se[:, b * T:(b + 1) * T],
                                     se1w_sb[:, kt * 64:(kt + 1) * 64],
                                     gp[:, b * 10 + kt:b * 10 + kt + T],
                                     start=(kt == 0), stop=(kt == 2))
            a1 = consts.tile([64, F], F32)
            nc.scalar.activation(a1[:], p1se[:], AF.Relu,
                                 bias=se1b_sb[:, 0:1], scale=1.0)
            a1p = consts.tile([64, 20], F32R)
            nc.vector.tensor_copy(a1p[:], ztiny[:64, 0:20])
            for b in range(B):
                nc.vector.tensor_copy(a1p[:, b * 10 + 1:b * 10 + 9],
                                      a1[:, b * T:(b + 1) * T])
            pev = ps_se.tile([DIM, F], F32, tag="se2e")
            pod = ps_se.tile([DIM, F], F32, tag="se2o")
            for b in range(B):
                for kt in range(K):
                    nc.tensor.matmul(pev[:, b * T:(b + 1) * T],
                                     se2w_sb[:, kt * 256:kt * 256 + DIM],
                                     a1p[:, b * 10 + kt:b * 10 + kt + T],
                                     start=(kt == 0), stop=(kt == 2))
                    nc.tensor.matmul(pod[:, b * T:(b + 1) * T],
                                     se2w_sb[:, kt * 256 + DIM:kt * 256 + 256],
                                     a1p[:, b * 10 + kt:b * 10 + kt + T],
                                     start=(kt == 0), stop=(kt == 2))
            pev_sb = consts.tile([DIM, F], F32)
            nc.vector.tensor_copy(pev_sb[:], pev[:])
            dse = consts.tile([DIM, F], F32)
            nc.vector.tensor_sub(dse[:], pev_sb[:], pod[:])
            nc.scalar.activation(attn0_sb[:], dse[:], AF.Sigmoid,
                                 bias=se2bd_sb[:, 0:1], scale=1.0)
            if debug:
                nc.sync.dma_start(attn_dbg[:, :], attn0_sb[:])

        stk2.close()

        # =========================================================
        # PHASE 2c: blend + output
        # =========================================================
        with tc.tile_pool(name="p2c", bufs=2) as p2c:
            for f in range(F):
                b, t = f // T, f % T
                yb = p2c.tile([DIM, LWV], BF16, tag="yb")
                kb = p2c.tile([DIM, LWV], BF16, tag="kb")
                nc.sync.dma_start(yb[:], y_dram[f])
                nc.sync.dma_start(kb[:], kf_dram[f])
                d2 = p2c.tile([DIM, LWV], BF16, tag="d2")
                nc.vector.tensor_sub(d2[:], yb[:], kb[:])
                ob = p2c.tile([DIM, LWV], F32, tag="ob")
                nc.vector.scalar_tensor_tensor(
                    ob[:], d2[:], attn0_sb[:, f:f + 1], kb[:], ALU.mult, ALU.add)
                src = ob[:].rearrange("p (r w) -> p r w", r=RWV)[:, :, 1:97]
                nc.sync.dma_start(out_d[b, :, t], src)

    nc.finalize()
    return nc


# =====================================================================
# host-side preparation
# =====================================================================

def _fold_bn(g, b, m, v):
    s = (np.asarray(g, np.float32) / np.sqrt(np.asarray(v, np.float32) + EPS))
    return (s.astype(np.float32),
            (np.asarray(b, np.float32) - np.asarray(m, np.float32) * s).astype(np.float32))


def prep_inputs(inp):
    f32 = np.float32
    x = np.asarray(inp["x"], f32)

    s_ke, b_ke = _fold_bn(inp["ke_g"], inp["ke_b"], inp["ke_m"], inp["ke_v"])
    s_e1, b_e1 = _fold_bn(inp["e1_g"], inp["e1_b"], inp["e1_m"], inp["e1_v"])
    s_c1, b_c1 = _fold_bn(inp["c1_g"], inp["c1_b"], inp["c1_m"], inp["c1_v"])
    s_b2, b_b2 = _fold_bn(inp["bn2_g"], inp["bn2_b"], inp["bn2_m"], inp["bn2_v"])
    s_s1, b_s1 = _fold_bn(inp["se1_g"], inp["se1_b"], inp["se1_m"], inp["se1_v"])

    kew = np.zeros((27, DIM, DIM), f32)
    KW = np.asarray(inp["ke_w"], f32) * s_ke[:, None, None, None, None]
    for kt in range(K):
        for kh in range(K):
            for kw_ in range(K):
                tap = (kt * K + kh) * K + kw_
                for g in range(4):
                    blk = KW[g * 32:(g + 1) * 32, :, kt, kh, kw_]
                    kew[tap, g * 32:(g + 1) * 32, g * 32:(g + 1) * 32] = blk.T
    kew = kew.transpose(1, 0, 2).reshape(DIM, 27 * DIM).copy()

    E1 = np.asarray(inp["e1_w"], f32) * s_e1[:, None, None, None, None]
    e1w = np.zeros((54, DIM, 64), f32)
    for kt in range(K):
        for kh in range(K):
            for kw_ in range(K):
                tap = (kt * K + kh) * K + kw_
                e1w[tap * 2] = E1[:, :DIM, kt, kh, kw_].T
                e1w[tap * 2 + 1] = E1[:, DIM:, kt, kh, kw_].T
    e1w = e1w.transpose(1, 0, 2).reshape(DIM, 54 * 64).copy()

    # e2 weights packed for frame-stacked moving operands:
    #  A:  [e1[f-1]; e1[f]] pairs -> rows 0:64 = kt0, rows 64:128 = kt1
    #  A1: kt1 only (first frame of a batch has no e1[f-1])
    #  B:  [e1[f+1]; e1[f+1]+WP] pairs -> kt2 with kh0 / kh1
    #  C:  kt2, kh2 singles
    E2 = np.asarray(inp["e2_w"], f32)
    e2wA = np.zeros((DIM, 9 * EMB), f32)
    e2wA1 = np.zeros((64, 9 * EMB), f32)
    for kh in range(K):
        for kw_ in range(K):
            idx = kh * K + kw_
            e2wA[0:64, idx * EMB:(idx + 1) * EMB] = E2[:, :, 0, kh, kw_].T
            e2wA[64:128, idx * EMB:(idx + 1) * EMB] = E2[:, :, 1, kh, kw_].T
            e2wA1[:, idx * EMB:(idx + 1) * EMB] = E2[:, :, 1, kh, kw_].T
    e2wB = np.zeros((DIM, 3 * EMB), f32)
    e2wC = np.zeros((64, 3 * EMB), f32)
    for kw_ in range(K):
        e2wB[0:64, kw_ * EMB:(kw_ + 1) * EMB] = E2[:, :, 2, 0, kw_].T
        e2wB[64:128, kw_ * EMB:(kw_ + 1) * EMB] = E2[:, :, 2, 1, kw_].T
        e2wC[:, kw_ * EMB:(kw_ + 1) * EMB] = E2[:, :, 2, 2, kw_].T

    C1 = np.asarray(inp["c1_w"], f32) * s_c1[:, None, None, None, None]
    c1w = np.zeros((27, DIM, DIM), f32)
    for kt in range(K):
        for kh in range(K):
            for kw_ in range(K):
                tap = (kt * K + kh) * K + kw_
                c1w[tap] = C1[:, :, kt, kh, kw_].T
    c1w = c1w.transpose(1, 0, 2).reshape(DIM, 27 * DIM).copy()

    S1 = np.asarray(inp["se1_w"], f32)[:, :, :, 1, 1] * s_s1[:, None, None]
    se1w = np.zeros((DIM, 3 * 64), f32)
    for kt in range(K):
        se1w[:, kt * 64:(kt + 1) * 64] = (S1[:, :, kt] / (H * W)).T
    se1b = (np.asarray(inp["se1_bias"], f32) * s_s1 + b_s1).reshape(64, 1)

    S2 = np.asarray(inp["se2_w"], f32)[:, :, :, 1, 1]
    se2w = np.zeros((64, 3 * 256), f32)
    for kt in range(K):
        se2w[:, kt * 256:kt * 256 + DIM] = S2[0::2, :, kt].T
        se2w[:, kt * 256 + DIM:kt * 256 + 256] = S2[1::2, :, kt].T
    se2b = np.asarray(inp["se2_bias"], f32)
    se2bd = (se2b[0::2] - se2b[1::2]).reshape(DIM, 1)

    Emat = np.zeros((16, DIM), f32)
    for c in range(DIM):
        Emat[c // SHARE, c] = 1.0
    Gmat = np.zeros((DIM, 16), f32)
    for ch in range(DIM):
        Gmat[ch, ch // 9] = 1.0
    Ghmat = np.zeros((16, 16), f32)
    for i_ in range(16):
        Ghmat[i_, (DIM + i_) // 9] = 1.0

    import ml_dtypes
    bf16 = ml_dtypes.bfloat16
    shared = dict(
        kew=kew, keb=b_ke.reshape(DIM, 1),
        e1w=e1w, e1b=b_e1.reshape(64, 1),
        e2wA=e2wA, e2wA1=e2wA1, e2wB=e2wB, e2wC=e2wC,
        e2b=np.asarray(inp["e2_bias"], f32).reshape(EMB, 1),
        c1w=c1w, c1b=b_c1.reshape(DIM, 1),
        bn2s=np.stack([s_b2, b_b2], axis=1).astype(f32),
        se1w=se1w, se1b=se1b, se2w=se2w, se2bd=se2bd,
        gng=np.asarray(inp["gn_g"], f32).reshape(16, 9),
        gnb=np.asarray(inp["gn_b"], f32).reshape(16, 9),
        Emat=Emat.astype(bf16), Ematf=Emat, Gmat=Gmat, Ghmat=Ghmat,
    )

    def mk_mask(parts, rows, base, dt=f32):
        m = np.zeros((parts, rows, WP), f32)
        for i in range(rows):
            if 0 <= base + i < H:
                m[:, i, 1:97] = 1.0
        return m.reshape(parts, rows * WP).astype(dt)

    per_core = []
    for c in range(NCORES):
        own0 = c * ROWN
        xc = np.zeros((F, DIM, RX, WP), f32)
        r0 = own0 - 3
        lo, hi = max(0, r0), min(H, r0 + RX)
        for b in range(B):
            xc[b * T:(b + 1) * T, :, lo - r0:hi - r0, 1:97] = \
                x[b].transpose(1, 0, 2, 3)[:, :, lo:hi, :]
        d = dict(shared)
        d["xin"] = xc.reshape(F, DIM, LX)
        d["kmask"] = mk_mask(DIM, RKF, own0 - 2, bf16)
        d["e1mask"] = mk_mask(64, RE1, own0 - 1, bf16)
        d["wvmask"] = mk_mask(DIM, RWV, own0)
        d["xqmask"] = mk_mask(DIM, RXQ, own0 - 1, bf16)
        per_core.append(d)
    return per_core


# =====================================================================
# resident runner
# =====================================================================
class _Runner:
    def __init__(self, nc, n_cores):
        import jax
        from jax.sharding import Mesh, PartitionSpec, NamedSharding
        from jax.experimental.shard_map import shard_map
        from concourse.bass2jax import (_bass_exec_p, install_neuronx_cc_hook,
                                        partition_id_tensor)
        install_neuronx_cc_hook()
        self.jax = jax
        self.nc = nc
        self.n_cores = n_cores

        in_names, out_names, out_avals, zero_outs = [], [], [], []
        pid_name = nc.partition_id_tensor.name if nc.partition_id_tensor else None
        for alloc in nc.m.functions[0].allocations:
            if not isinstance(alloc, mybir.MemoryLocationSet):
                continue
            name = alloc.memorylocations[0].name
            if alloc.kind == "ExternalInput":
                if name != pid_name:
                    in_names.append(name)
            elif alloc.kind == "ExternalOutput":
                out_names.append(name)
                out_avals.append(jax.core.ShapedArray(
                    tuple(alloc.tensor_shape), mybir.dt.np(alloc.dtype)))
                zero_outs.append(np.zeros(tuple(alloc.tensor_shape),
                                          mybir.dt.np(alloc.dtype)))
        self.in_names, self.out_names = in_names, out_names
        self.out_avals, self.zero_outs = out_avals, zero_outs
        n_params = len(in_names)
        all_in_names = list(in_names) + list(out_names)
        if pid_name is not None:
            all_in_names.append(pid_name)
        has_pid = pid_name is not None

        def _body(*args):
            operands = list(args)
            if has_pid:
                operands.append(partition_id_tensor())
            return tuple(_bass_exec_p.bind(
                *operands,
                out_avals=tuple(out_avals),
                in_names=tuple(all_in_names),
                out_names=tuple(out_names),
                lowering_input_output_aliases=(),
                sim_require_finite=True,
                sim_require_nnan=True,
                nc=nc,
            ))

        devices = jax.devices()[:n_cores]
        self.mesh = Mesh(np.asarray(devices), ("core",))
        self.sharding = NamedSharding(self.mesh, PartitionSpec("core"))
        in_specs = (PartitionSpec("core"),) * (n_params + len(out_names))
        out_specs = (PartitionSpec("core"),) * len(out_names)
        self.fn = jax.jit(
            shard_map(_body, mesh=self.mesh, in_specs=in_specs,
                      out_specs=out_specs, check_rep=False),
            keep_unused=True)
        self._zero_dev = None

    def put_inputs(self, in_maps):
        jax = self.jax
        concat = [np.concatenate([np.asarray(in_maps[c][n])
                                  for c in range(self.n_cores)], axis=0)
                  for n in self.in_names]
        if self._zero_dev is None:
            zeros = [np.concatenate([z] * self.n_cores, axis=0)
                     for z in self.zero_outs]
            self._zero_dev = [jax.device_put(z, self.sharding) for z in zeros]
        self._dev_in = [jax.device_put(a, self.sharding) for a in concat]
        jax.block_until_ready(self._dev_in)

    def run(self):
        return self.fn(*self._dev_in, *self._zero_dev)

    def run_np(self):
        jax = self.jax
        out = jax.block_until_ready(self.run())
        res = []
        for c in range(self.n_cores):
            d = {}
            for i, name in enumerate(self.out_names):
                full = np.asarray(out[i])
                d[name] = full.reshape(self.n_cores, *self.out_avals[i].shape)[c]
            res.append(d)
        return res


_CACHE = {}


def _get_runner(debug=False):
    key = ("runner", debug)
    if key not in _CACHE:
        nc = build_nc(debug=debug)
        _CACHE[key] = _Runner(nc, NCORES)
    return _CACHE[key]


def kernel(**inputs) -> np.ndarray:
    per_core = prep_inputs(inputs)
    r = _get_runner(debug=False)
    r.put_inputs(per_core)
    res = r.run_np()
    out = np.concatenate([res[c]["out"] for c in range(NCORES)], axis=3)
    return out.astype(np.float32)



# revision 30
# speedup vs baseline: 1.3317x; 1.0272x over previous
"""Trainium2 Bass kernel for nn_CotLayer (CoT attention layer).

Strategy: H-dimension sharding across 8 NeuronCores (12 of 96 rows each, all
B*T frames per core, halo rows included host-side).  All 3x3x3 convs are
implicit GEMM: channels on partitions, padded-width (98) pixel lines on the
free dim, 27 shifted matmuls accumulating in PSUM (fp32r = full PE rate).
GroupNorm statistics and the split-attention gap are AllReduced across cores.

Self-contained: only numpy/jax/concourse imports.
"""
from contextlib import ExitStack

import numpy as np

import concourse.bacc as bacc
import concourse.tile as tile
from concourse import mybir

F32 = mybir.dt.float32
F32R = mybir.dt.float32r
BF16 = mybir.dt.bfloat16
AF = mybir.ActivationFunctionType
ALU = mybir.AluOpType

# problem constants
B, DIM, T, H, W = 2, 128, 8, 96, 96
K, SHARE, RADIX = 3, 8, 2
EMB = 144
EPS = 1e-5
NCORES = 8
F = B * T               # 16 frames
ROWN = H // NCORES      # 12 owned rows per core
WP = W + 2              # padded width

# slab geometry (rows, line length)
RX, RKF, RE1, RWV, RXQ = 18, 16, 14, 12, 14
LX, LKF, LE1, LWV, LXQ = RX * WP, RKF * WP, RE1 * WP, RWV * WP, RXQ * WP

GN_CNT = float((EMB // 16) * T * H * W)   # 9*8*96*96 per (b, group)


def _chunks(L, n):
    # fp32r matmul requires even moving-operand counts -> even chunk sizes
    assert L % 2 == 0
    base2 = (L // n) // 2 * 2
    rem = L - base2 * n
    out, s = [], 0
    for i in range(n):
        e = s + base2 + (2 if i < rem // 2 else 0)
        out.append((s, e))
        s = e
    assert s == L
    return out


CH_KF = _chunks(LKF, 4)
CH_E1 = _chunks(LE1, 3)
CH_WV = _chunks(LWV, 3)
CH_XQ = _chunks(LXQ, 3)


def _same_batch(f, g):
    return 0 <= g < F and g // T == f // T


def build_nc(debug=False, ncores=NCORES):
    nc = bacc.Bacc()

    # ---------------- I/O ----------------
    xin = nc.dram_tensor("xin", [F, DIM, LX], BF16, kind="ExternalInput")
    kew = nc.dram_tensor("kew", [DIM, 27 * DIM], BF16, kind="ExternalInput")
    keb = nc.dram_tensor("keb", [DIM, 1], F32, kind="ExternalInput")
    e1w = nc.dram_tensor("e1w", [DIM, 54 * 64], BF16, kind="ExternalInput")
    e1b = nc.dram_tensor("e1b", [64, 1], F32, kind="ExternalInput")
    e2wA = nc.dram_tensor("e2wA", [DIM, 9 * EMB], BF16, kind="ExternalInput")
    e2wA1 = nc.dram_tensor("e2wA1", [64, 9 * EMB], BF16, kind="ExternalInput")
    e2wB = nc.dram_tensor("e2wB", [DIM, 3 * EMB], BF16, kind="ExternalInput")
    e2wC = nc.dram_tensor("e2wC", [64, 3 * EMB], BF16, kind="ExternalInput")
    e2b = nc.dram_tensor("e2b", [EMB, 1], F32, kind="ExternalInput")
    c1w = nc.dram_tensor("c1w", [DIM, 27 * DIM], BF16, kind="ExternalInput")
    c1b = nc.dram_tensor("c1b", [DIM, 1], F32, kind="ExternalInput")
    bn2s = nc.dram_tensor("bn2s", [DIM, 2], F32, kind="ExternalInput")
    se1w_d = nc.dram_tensor("se1w", [DIM, 3 * 64], F32R, kind="ExternalInput")
    se1b_d = nc.dram_tensor("se1b", [64, 1], F32, kind="ExternalInput")
    se2w_d = nc.dram_tensor("se2w", [64, 3 * 256], F32R, kind="ExternalInput")
    se2bd_d = nc.dram_tensor("se2bd", [DIM, 1], F32, kind="ExternalInput")
    gng_d = nc.dram_tensor("gng", [16, 9], F32, kind="ExternalInput")
    gnb_d = nc.dram_tensor("gnb", [16, 9], F32, kind="ExternalInput")
    E_d = nc.dram_tensor("Emat", [16, DIM], BF16, kind="ExternalInput")
    Ef_d = nc.dram_tensor("Ematf", [16, DIM], F32R, kind="ExternalInput")
    G_d = nc.dram_tensor("Gmat", [DIM, 16], F32, kind="ExternalInput")
    Gh_d = nc.dram_tensor("Ghmat", [16, 16], F32, kind="ExternalInput")
    kmask_d = nc.dram_tensor("kmask", [DIM, LKF], BF16, kind="ExternalInput")
    e1mask_d = nc.dram_tensor("e1mask", [64, LE1], BF16, kind="ExternalInput")
    wvmask_d = nc.dram_tensor("wvmask", [DIM, LWV], F32, kind="ExternalInput")
    xqmask_d = nc.dram_tensor("xqmask", [DIM, LXQ], BF16, kind="ExternalInput")

    out_d = nc.dram_tensor("out", [B, DIM, T, ROWN, W], F32, kind="ExternalOutput")

    dbg = "ExternalOutput" if debug else "Internal"
    wv_dram = nc.dram_tensor("wv_s", [F, EMB, LWV], BF16, kind=dbg)
    xq_dram = nc.dram_tensor("xq_s", [F, DIM, LXQ], BF16, kind=dbg)
    kf_dram = nc.dram_tensor("kf_s", [F, DIM, LWV], BF16, kind=dbg)
    y_dram = nc.dram_tensor("y_s", [F, DIM, LWV], BF16, kind=dbg)
    if debug:
        attn_dbg = nc.dram_tensor("attn_dbg", [DIM, 16], F32, kind="ExternalOutput")
        gstat_dbg = nc.dram_tensor("gstat_dbg", [16, 4], F32, kind="ExternalOutput")

    shr = {} if ncores == 1 else {"addr_space": "Shared"}
    ar1_in = nc.dram_tensor("ar1_in", [16, 4], F32)
    ar1_out = nc.dram_tensor("ar1_out", [16, 4], F32, **shr)
    ar2_in = nc.dram_tensor("ar2_in", [DIM, F], F32)
    ar2_out = nc.dram_tensor("ar2_out", [DIM, F], F32, **shr)
    RG = [list(range(ncores))]

    with tile.TileContext(nc) as tc, ExitStack() as stk:
        consts = stk.enter_context(tc.tile_pool(name="consts", bufs=1))

        def load_const(dram, p, l, dt=F32):
            t = consts.tile([p, l], dt, name=dram.name + "_sb")
            nc.sync.dma_start(t[:], dram[:, :])
            return t

        kew_sb = load_const(kew, DIM, 27 * DIM, BF16)
        e1w_sb = load_const(e1w, DIM, 54 * 64, BF16)
        e2wA_sb = load_const(e2wA, DIM, 9 * EMB, BF16)
        e2wA1_sb = load_const(e2wA1, 64, 9 * EMB, BF16)
        e2wB_sb = load_const(e2wB, DIM, 3 * EMB, BF16)
        e2wC_sb = load_const(e2wC, 64, 3 * EMB, BF16)
        c1w_sb = load_const(c1w, DIM, 27 * DIM, BF16)
        se1w_sb = load_const(se1w_d, DIM, 3 * 64, F32R)
        se2w_sb = load_const(se2w_d, 64, 3 * 256, F32R)
        E_sb = load_const(E_d, 16, DIM, BF16)
        Ef_sb = load_const(Ef_d, 16, DIM, F32R)
        G_sb = load_const(G_d, DIM, 16)
        Gh_sb = load_const(Gh_d, 16, 16)
        keb_sb = load_const(keb, DIM, 1)
        e1b_sb = load_const(e1b, 64, 1)
        c1b_sb = load_const(c1b, DIM, 1)
        bn2_sb = load_const(bn2s, DIM, 2)
        se1b_sb = load_const(se1b_d, 64, 1)
        se2bd_sb = load_const(se2bd_d, DIM, 1)
        gng_sb = load_const(gng_d, 16, 9)
        gnb_sb = load_const(gnb_d, 16, 9)
        wvmask_sb = load_const(wvmask_d, DIM, LWV)
        e2b_lo_sb = consts.tile([DIM, 1], F32)
        nc.sync.dma_start(e2b_lo_sb[:], e2b[0:DIM, :])
        e2b_hi_sb = consts.tile([16, 1], F32)
        nc.sync.dma_start(e2b_hi_sb[:], e2b[DIM:EMB, :])

        ztiny = consts.tile([DIM, 24], F32)
        nc.vector.memset(ztiny[:], 0.0)

        # stats / gap accumulators
        ssum_lo = consts.tile([DIM, 3 * F], F32)
        ssq_lo = consts.tile([DIM, 3 * F], F32)
        ssum_hi = consts.tile([16, 3 * F], F32)
        ssq_hi = consts.tile([16, 3 * F], F32)
        gap_sb = consts.tile([DIM, F], F32)
        gapy_sb = consts.tile([DIM, F], F32)
        gstat_sb = consts.tile([16, 4], F32)
        scale_sb = consts.tile([16, 2 * 9], F32)
        bias_sb = consts.tile([16, 2 * 9], F32)
        attn0_sb = consts.tile([DIM, F], F32)
        sred_lo = consts.tile([DIM, 4], F32)
        sred_hi = consts.tile([16, 4], F32)

        # =========================================================
        # PHASE 1
        # =========================================================
        stk1 = ExitStack()
        p1 = stk1.enter_context(tc.tile_pool(name="p1", bufs=1))
        p1stage = stk1.enter_context(tc.tile_pool(name="p1stage", bufs=2))
        stk1ps = ExitStack()
        ps_ke = stk1ps.enter_context(tc.tile_pool(name="ps_ke", bufs=2, space="PSUM"))
        ps_e1 = stk1ps.enter_context(tc.tile_pool(name="ps_e1", bufs=2, space="PSUM"))
        ps_e2l = stk1ps.enter_context(tc.tile_pool(name="ps_e2l", bufs=2, space="PSUM"))
        ps_e2h = stk1ps.enter_context(tc.tile_pool(name="ps_e2h", bufs=2, space="PSUM"))

        kmask_sb = p1.tile([DIM, LKF], BF16)
        nc.sync.dma_start(kmask_sb[:], kmask_d[:, :])
        e1mask_sb = p1.tile([64, LE1], BF16)
        nc.sync.dma_start(e1mask_sb[:], e1mask_d[:, :])
        xqmask_sb = p1.tile([DIM, LXQ], BF16)
        nc.sync.dma_start(xqmask_sb[:], xqmask_d[:, :])

        zslab = p1.tile([DIM, LX + 2], BF16)
        zsf = p1.tile([DIM, LX + 2], F32)
        nc.vector.memset(zsf[:], 0.0)
        nc.vector.tensor_copy(zslab[:], zsf[:])

        x_ring = [p1.tile([DIM, LX + 2], BF16, tag=f"xr{i}", name=f"xr{i}")
                  for i in range(4)]
        kf_ring = [p1.tile([DIM, LKF + 2], BF16, tag=f"kfr{i}", name=f"kfr{i}")
                   for i in range(3)]
        # e2 frame-stacked moving tiles:
        #  pair_ring[f%3]: rows 0:64 = e1[f-1], rows 64:128 = e1[f]
        #  s2x_ring[f%2]:  rows 0:64 = e1[f+1], rows 64:128 = e1[f+1] shifted +WP
        pair_ring = [p1.tile([DIM, LE1 + 2], BF16, tag=f"pr{i}", name=f"pr{i}")
                     for i in range(3)]
        s2x_ring = [p1.tile([DIM, LE1 + 2], BF16, tag=f"s2x{i}", name=f"s2x{i}")
                    for i in range(2)]
        edge_t = p1.tile([64, LE1 + 2], BF16, name="edge_t")
        for t_ in pair_ring + s2x_ring + [edge_t]:
            pp = t_.shape[0]
            nc.vector.tensor_copy(t_[:], zsf[:pp, 0:LE1 + 2])
        for t_ in x_ring + kf_ring:
            pp = t_.shape[0]
            L = t_.shape[1]
            nc.vector.tensor_copy(t_[:, 0:1], zslab[:pp, 0:1])
            nc.vector.tensor_copy(t_[:, L - 1:L], zslab[:pp, 0:1])

        def load_x(f):
            nc.sync.dma_start(x_ring[f % 4][:, 1:LX + 1], xin[f])

        def xsrc(f, d):
            return x_ring[(f + d) % 4] if _same_batch(f, f + d) else zslab

        def kfsrc(f, d):
            return kf_ring[(f + d) % 3] if _same_batch(f, f + d) else zslab

        # ---------------- PHASE 1a ----------------
        load_x(0)
        for i in range(F + 2):
            if i + 1 < F:
                load_x(i + 1)
            # A: kf[i]
            if i < F:
                kf_t = kf_ring[i % 3]
                for (q0, q1) in CH_KF:
                    n = q1 - q0
                    p = ps_ke.tile([DIM, n], F32, tag="ke")
                    for kt in range(K):
                        src = xsrc(i, kt - 1)
                        for kh in range(K):
                            for kw in range(K):
                                tap = (kt * K + kh) * K + kw
                                off = 1 + q0 + kh * WP + kw - 1
                                nc.tensor.matmul(
                                    p[:], kew_sb[:, tap * DIM:(tap + 1) * DIM],
                                    src[:, off:off + n],
                                    start=(tap == 0), stop=(tap == 26))
                    nc.scalar.activation(kf_t[:, 1 + q0:1 + q1], p[:], AF.Relu,
                                         bias=keb_sb[:, 0:1], scale=1.0)
                nc.vector.tensor_mul(kf_t[:, 1:LKF + 1], kf_t[:, 1:LKF + 1],
                                     kmask_sb[:])
                kstore = p1stage.tile([DIM, LWV], BF16, tag="kfstore")
                own = kf_t[:, 1:LKF + 1].rearrange("p (r w) -> p r w", r=RKF)[:, 2:14, :]
                nc.scalar.activation(
                    kstore[:].rearrange("p (r w) -> p r w", r=RWV), own,
                    AF.Identity, bias=0.0, scale=1.0,
                    accum_out=gap_sb[:, i:i + 1])
                nc.sync.dma_start(kf_dram[i], kstore[:])
            # B: e1out[i-1] (before x[i+1] prefetch overwrites x[i-2])
            tb = i - 1
            if 0 <= tb < F:
                e1_t = p1stage.tile([64, LE1], BF16, tag="e1st")
                for (q0, q1) in CH_E1:
                    n = q1 - q0
                    p = ps_e1.tile([64, n], F32, tag="e1")
                    for kt in range(K):
                        sx = xsrc(tb, kt - 1)
                        sk = kfsrc(tb, kt - 1)
                        for kh in range(K):
                            for kw in range(K):
                                tap = (kt * K + kh) * K + kw
                                offx = 1 + q0 + (kh + 1) * WP + kw - 1
                                offk = 1 + q0 + kh * WP + kw - 1
                                c0 = (tap * 2) * 64
                                nc.tensor.matmul(
                                    p[:], e1w_sb[:, c0:c0 + 64],
                                    sx[:, offx:offx + n],
                                    start=(tap == 0), stop=False)
                                nc.tensor.matmul(
                                    p[:], e1w_sb[:, c0 + 64:c0 + 128],
                                    sk[:, offk:offk + n],
                                    start=False, stop=(tap == 26))
                    nc.scalar.activation(e1_t[:, q0:q1], p[:], AF.Relu,
                                         bias=e1b_sb[:, 0:1], scale=1.0)
                nc.vector.tensor_mul(e1_t[:], e1_t[:], e1mask_sb[:])
                # scatter into the frame-stacked e2 moving tiles
                nc.sync.dma_start(pair_ring[tb % 3][64:128, 1:1 + LE1], e1_t[:])
                if tb % T != T - 1:
                    nc.sync.dma_start(pair_ring[(tb + 1) % 3][0:64, 1:1 + LE1],
                                      e1_t[:])
                if tb % T != 0:
                    s2w = s2x_ring[(tb - 1) % 2]
                    nc.sync.dma_start(s2w[0:64, 1:1 + LE1], e1_t[:])
                    nc.sync.dma_start(s2w[64:128, 0:LE1 - WP + 1],
                                      e1_t[:, WP - 1:LE1])
                else:
                    nc.sync.dma_start(edge_t[:, 1:1 + LE1], e1_t[:])
            # C: wv[i-2]  (e2 conv, frame-stacked: <=15 streams/chunk/half)
            f2 = i - 2
            if 0 <= f2 < F:
                t_in_b = f2 % T
                s2 = s2x_ring[f2 % 2]
                n_mm = 9 + (6 if t_in_b != T - 1 else 0)
                wlo = p1stage.tile([DIM, LWV], BF16, tag="wvlo")
                whi = p1stage.tile([16, LWV], BF16, tag="wvhi")
                sqj = p1stage.tile([DIM, 392], F32, tag="sqjunk")
                for ci, (q0, q1) in enumerate(CH_WV):
                    n = q1 - q0
                    plo = ps_e2l.tile([DIM, n], F32, tag="e2l")
                    phi = ps_e2h.tile([16, n], F32, tag="e2h")
                    mm = 0

                    def emit(w_sb, c0, mv_sl, nmm=n_mm, plo=plo, phi=phi):
                        nonlocal mm
                        nc.tensor.matmul(plo[:], w_sb[:, c0:c0 + DIM], mv_sl,
                                         start=(mm == 0), stop=(mm == nmm - 1))
                        nc.tensor.matmul(phi[:], w_sb[:, c0 + DIM:c0 + EMB],
                                         mv_sl,
                                         start=(mm == 0), stop=(mm == nmm - 1))
                        mm += 1

                    for kh in range(K):
                        for kw in range(K):
                            c0 = (kh * K + kw) * EMB
                            off = 1 + q0 + kh * WP + kw - 1
                            if t_in_b == 0:
                                emit(e2wA1_sb, c0, edge_t[:, off:off + n])
                            else:
                                emit(e2wA_sb, c0,
                                     pair_ring[f2 % 3][:, off:off + n])
                    if t_in_b != T - 1:
                        for kw in range(K):
                            off = 1 + q0 + kw - 1
                            emit(e2wB_sb, kw * EMB, s2[:, off:off + n])
                        for kw in range(K):
                            off = 1 + q0 + 2 * WP + kw - 1
                            emit(e2wC_sb, kw * EMB, s2[0:64, off:off + n])
                    assert mm == n_mm
                    col = f2 * 3 + ci
                    nc.vector.scalar_tensor_tensor(
                        wlo[:, q0:q1], plo[:], e2b_lo_sb[:, 0:1],
                        wvmask_sb[:, q0:q1],
                        ALU.add, ALU.mult, accum_out=ssum_lo[:, col:col + 1])
                    nc.vector.scalar_tensor_tensor(
                        whi[:, q0:q1], phi[:], e2b_hi_sb[:, 0:1],
                        wvmask_sb[:16, q0:q1],
                        ALU.add, ALU.mult, accum_out=ssum_hi[:, col:col + 1])
                    nc.scalar.activation(sqj[:, :n], wlo[:, q0:q1], AF.Square,
                                         bias=0.0, scale=1.0,
                                         accum_out=ssq_lo[:, col:col + 1])
                    nc.scalar.activation(sqj[:16, :n], whi[:, q0:q1], AF.Square,
                                         bias=0.0, scale=1.0,
                                         accum_out=ssq_hi[:, col:col + 1])
                nc.sync.dma_start(wv_dram[f2, 0:DIM], wlo[:])
                nc.sync.dma_start(wv_dram[f2, DIM:EMB], whi[:])

        # ---- GN stats reduce + AllReduce ----
        stk1ps.close()
        half = 3 * T
        nc.vector.reduce_sum(sred_lo[:, 0:1], ssum_lo[:, 0:half], axis=mybir.AxisListType.X)
        nc.vector.reduce_sum(sred_lo[:, 1:2], ssum_lo[:, half:], axis=mybir.AxisListType.X)
        nc.vector.reduce_sum(sred_lo[:, 2:3], ssq_lo[:, 0:half], axis=mybir.AxisListType.X)
        nc.vector.reduce_sum(sred_lo[:, 3:4], ssq_lo[:, half:], axis=mybir.AxisListType.X)
        nc.vector.reduce_sum(sred_hi[:, 0:1], ssum_hi[:, 0:half], axis=mybir.AxisListType.X)
        nc.vector.reduce_sum(sred_hi[:, 1:2], ssum_hi[:, half:], axis=mybir.AxisListType.X)
        nc.vector.reduce_sum(sred_hi[:, 2:3], ssq_hi[:, 0:half], axis=mybir.AxisListType.X)
        nc.vector.reduce_sum(sred_hi[:, 3:4], ssq_hi[:, half:], axis=mybir.AxisListType.X)
        with tc.tile_pool(name="ps_st", bufs=1, space="PSUM") as ps_st:
            pst = ps_st.tile([16, 4], F32, tag="gstat")
            nc.tensor.matmul(pst[:], G_sb[:], sred_lo[:], start=True, stop=False)
            nc.tensor.matmul(pst[:], Gh_sb[:], sred_hi[:], start=False, stop=True)
            gloc = consts.tile([16, 4], F32)
            nc.vector.tensor_copy(gloc[:], pst[:])
        nc.sync.dma_start(ar1_in[:, :], gloc[:])
        if ncores == 1:   # sim-profiling build: skip collectives
            nc.sync.dma_start(ar1_out[:, :], ar1_in[:, :])
        else:
            nc.gpsimd.collective_compute(
                "AllReduce", ALU.add, replica_groups=RG,
                ins=[ar1_in[:, :]], outs=[ar1_out[:, :]])
        nc.sync.dma_start(gstat_sb[:], ar1_out[:, :])
        if debug:
            nc.sync.dma_start(gstat_dbg[:, :], gstat_sb[:])

        # ---------------- PHASE 1b: c1 (overlaps AR1) ----------------
        with tc.tile_pool(name="ps_c1", bufs=3, space="PSUM") as ps_c1:
            load_x(0)
            for f in range(F):
                if f + 1 < F:
                    load_x(f + 1)
                xst = p1stage.tile([DIM, LXQ], BF16, tag="xqstage")
                for (q0, q1) in CH_XQ:
                    n = q1 - q0
                    p = ps_c1.tile([DIM, n], F32, tag="c1")
                    for kt in range(K):
                        src = xsrc(f, kt - 1)
                        for kh in range(K):
                            for kw in range(K):
                                tap = (kt * K + kh) * K + kw
                                off = 1 + q0 + (kh + 1) * WP + kw - 1
                                nc.tensor.matmul(
                                    p[:], c1w_sb[:, tap * DIM:(tap + 1) * DIM],
                                    src[:, off:off + n],
                                    start=(tap == 0), stop=(tap == 26))
                    nc.vector.scalar_tensor_tensor(
                        xst[:, q0:q1], p[:], c1b_sb[:, 0:1], xqmask_sb[:, q0:q1],
                        ALU.add, ALU.mult)
                nc.sync.dma_start(xq_dram[f], xst[:])

        # ---- GN scale/bias ----
        mu = consts.tile([16, 2], F32)
        msq = consts.tile([16, 2], F32)
        mu2 = consts.tile([16, 2], F32)
        var = consts.tile([16, 2], F32)
        sd = consts.tile([16, 2], F32)
        rsq = consts.tile([16, 2], F32)
        tmp9 = consts.tile([16, 9], F32)
        nc.vector.tensor_scalar(mu[:], gstat_sb[:, 0:2], 1.0 / GN_CNT, None, ALU.mult)
        nc.vector.tensor_scalar(msq[:], gstat_sb[:, 2:4], 1.0 / GN_CNT, None, ALU.mult)
        nc.vector.tensor_mul(mu2[:], mu[:], mu[:])
        nc.vector.tensor_sub(var[:], msq[:], mu2[:])
        nc.vector.tensor_scalar(var[:], var[:], EPS, None, ALU.add)
        nc.scalar.activation(sd[:], var[:], AF.Sqrt, bias=0.0, scale=1.0)
        nc.vector.reciprocal(rsq[:], sd[:])
        for b in range(B):
            nc.vector.tensor_scalar(scale_sb[:, b * 9:(b + 1) * 9], gng_sb[:],
                                    rsq[:, b:b + 1], None, ALU.mult)
            nc.vector.tensor_scalar(tmp9[:], scale_sb[:, b * 9:(b + 1) * 9],
                                    mu[:, b:b + 1], None, ALU.mult)
            nc.vector.tensor_sub(bias_sb[:, b * 9:(b + 1) * 9], gnb_sb[:], tmp9[:])

        stk1.close()

        # =========================================================
        # PHASE 2a: local conv + bn2/swish + gap partials
        # =========================================================
        stk2 = ExitStack()
        p2 = stk2.enter_context(tc.tile_pool(name="p2", bufs=1))
        p2stage = stk2.enter_context(tc.tile_pool(name="p2stage", bufs=2))
        ps_ex = stk2.enter_context(tc.tile_pool(name="ps_ex", bufs=4, space="PSUM"))

        # GN affine folded into the broadcast: ek = E * scale(b,k) (stationary),
        # tbias[c, b*9+k] = bias(g(c), b, k) applied via the stt scalar.
        ek_sb = p2.tile([16, 2 * 9 * DIM], BF16, name="ek")
        for b in range(B):
            for k in range(9):
                j = b * 9 + k
                nc.vector.tensor_scalar(ek_sb[:, j * DIM:(j + 1) * DIM], E_sb[:],
                                        scale_sb[:, j:j + 1], None, ALU.mult)
        biasr = p2.tile([16, 2 * 9], F32R)
        nc.vector.tensor_copy(biasr[:], bias_sb[:])
        tbias = p2.tile([DIM, 2 * 9], F32)
        with tc.tile_pool(name="ps_tb", bufs=1, space="PSUM") as ps_tb:
            ptb = ps_tb.tile([DIM, 2 * 9], F32, tag="tb")
            nc.tensor.matmul(ptb[:], Ef_sb[:], biasr[:], start=True, stop=True)
            nc.vector.tensor_copy(tbias[:], ptb[:])

        xq_ring = [p2.tile([DIM, LXQ + 2], BF16, tag=f"xq{i}", name=f"xq{i}")
                   for i in range(2)]
        for t_ in xq_ring:
            nc.vector.memset(t_[:, 0:1], 0.0)
            nc.vector.memset(t_[:, LXQ + 1:LXQ + 2], 0.0)

        for f in range(F):
            b = f // T
            xq_t = xq_ring[f % 2]
            nc.sync.dma_start(xq_t[:, 1:LXQ + 1], xq_dram[f])
            wvr = p2stage.tile([16, 9 * LWV], BF16, tag="wvraw")
            nc.sync.dma_start(
                wvr[:].rearrange("p (k l) -> p k l", k=9),
                wv_dram[f].rearrange("(g k) l -> g k l", g=16))
            yacc = p2stage.tile([DIM, LWV], BF16, tag="yacc")
            tmpm = p2stage.tile([DIM, 392], BF16, tag="tmpm")
            peb = p2stage.tile([DIM, 9 * 392], BF16, tag="peb")
            for ci, (q0, q1) in enumerate(CH_WV):
                n = q1 - q0
                for k in range(9):
                    j = b * 9 + k
                    pe = ps_ex.tile([DIM, n], F32, tag="ex")
                    nc.tensor.matmul(pe[:], ek_sb[:, j * DIM:(j + 1) * DIM],
                                     wvr[:, k * LWV + q0:k * LWV + q1],
                                     start=True, stop=True)
                    nc.scalar.activation(peb[:, k * n:(k + 1) * n], pe[:],
                                         AF.Identity, bias=0.0, scale=1.0)
                for k in range(9):
                    dh, dw = k // 3, k % 3
                    j = b * 9 + k
                    off = 1 + q0 + dh * WP + dw - 1
                    if k == 0:
                        nc.vector.scalar_tensor_tensor(
                            yacc[:, q0:q1], peb[:, 0:n], tbias[:, j:j + 1],
                            xq_t[:, off:off + n], ALU.add, ALU.mult)
                    else:
                        nc.vector.scalar_tensor_tensor(
                            tmpm[:, :n], peb[:, k * n:(k + 1) * n],
                            tbias[:, j:j + 1],
                            xq_t[:, off:off + n], ALU.add, ALU.mult)
                        nc.vector.tensor_add(yacc[:, q0:q1], yacc[:, q0:q1],
                                             tmpm[:, :n])
            ysw = p2stage.tile([DIM, LWV], BF16, tag="ysw")
            nc.scalar.activation(ysw[:], yacc[:],
                                 AF.Tanh if ncores == 1 else AF.Silu,
                                 bias=bn2_sb[:, 1:2], scale=bn2_sb[:, 0:1])
            ym = p2stage.tile([DIM, LWV], BF16, tag="ym")
            nc.vector.scalar_tensor_tensor(
                ym[:], ysw[:], 1.0, wvmask_sb[:],
                ALU.mult, ALU.mult, accum_out=gapy_sb[:, f:f + 1])
            nc.sync.dma_start(y_dram[f], ym[:])

        # ---- gap AllReduce ----
        nc.vector.tensor_add(gap_sb[:], gap_sb[:], gapy_sb[:])
        nc.sync.dma_start(ar2_in[:, :], gap_sb[:])
        if ncores == 1:   # sim-profiling build: skip collectives
            nc.sync.dma_start(ar2_out[:, :], ar2_in[:, :])
        else:
            nc.gpsimd.collective_compute(
                "AllReduce", ALU.add, replica_groups=RG,
                ins=[ar2_in[:, :]], outs=[ar2_out[:, :]])
        gap_all = consts.tile([DIM, F], F32)
        nc.sync.dma_start(gap_all[:], ar2_out[:, :])

        # ---- SE block ----
        with tc.tile_pool(name="ps_se", bufs=1, space="PSUM") as ps_se:
            gp = consts.tile([DIM, 20], F32R)
            nc.vector.tensor_copy(gp[:], ztiny[:, 0:20])
            for b in range(B):
                nc.vector.tensor_copy(gp[:, b * 10 + 1:b * 10 + 9],
                                      gap_all[:, b * T:(b + 1) * T])
            p1se = ps_se.tile([64, F], F32, tag="se1")
            for b in range(B):
                for kt in range(K):
                    nc.tensor.matmul(p1se[:, b * T:(b + 1) * T],
                                     se1w_sb[:, kt * 64:(kt + 1) * 64],
                                     gp[:, b * 10 + kt:b * 10 + kt + T],
                                     start=(kt == 0), stop=(kt == 2))
            a1 = consts.tile([64, F], F32)
            nc.scalar.activation(a1[:], p1se[:], AF.Relu,
                                 bias=se1b_sb[:, 0:1], scale=1.0)
            a1p = consts.tile([64, 20], F32R)
            nc.vector.tensor_copy(a1p[:], ztiny[:64, 0:20])
            for b in range(B):
                nc.vector.tensor_copy(a1p[:, b * 10 + 1:b * 10 + 9],
                                      a1[:, b * T:(b + 1) * T])
            pev = ps_se.tile([DIM, F], F32, tag="se2e")
            pod = ps_se.tile([DIM, F], F32, tag="se2o")
            for b in range(B):
                for kt in range(K):
                    nc.tensor.matmul(pev[:, b * T:(b + 1) * T],
                                     se2w_sb[:, kt * 256:kt * 256 + DIM],
                                     a1p[:, b * 10 + kt:b * 10 + kt + T],
                                     start=(kt == 0), stop=(kt == 2))
                    nc.tensor.matmul(pod[:, b * T:(b + 1) * T],
                                     se2w_sb[:, kt * 256 + DIM:kt * 256 + 256],
                                     a1p[:, b * 10 + kt:b * 10 + kt + T],
                                     start=(kt == 0), stop=(kt == 2))
            pev_sb = consts.tile([DIM, F], F32)
            nc.vector.tensor_copy(pev_sb[:], pev[:])
            dse = consts.tile([DIM, F], F32)
            nc.vector.tensor_sub(dse[:], pev_sb[:], pod[:])
            nc.scalar.activation(attn0_sb[:], dse[:], AF.Sigmoid,
                                 bias=se2bd_sb[:, 0:1], scale=1.0)
            if debug:
                nc.sync.dma_start(attn_dbg[:, :], attn0_sb[:])

        stk2.close()

        # =========================================================
        # PHASE 2c: blend + output
        # =========================================================
        with tc.tile_pool(name="p2c", bufs=2) as p2c:
            for f in range(F):
                b, t = f // T, f % T
                yb = p2c.tile([DIM, LWV], BF16, tag="yb")
                kb = p2c.tile([DIM, LWV], BF16, tag="kb")
                nc.sync.dma_start(yb[:], y_dram[f])
                nc.sync.dma_start(kb[:], kf_dram[f])
                d2 = p2c.tile([DIM, LWV], BF16, tag="d2")
                nc.vector.tensor_sub(d2[:], yb[:], kb[:])
                ob = p2c.tile([DIM, LWV], F32, tag="ob")
                nc.vector.scalar_tensor_tensor(
                    ob[:], d2[:], attn0_sb[:, f:f + 1], kb[:], ALU.mult, ALU.add)
                src = ob[:].rearrange("p (r w) -> p r w", r=RWV)[:, :, 1:97]
                nc.sync.dma_start(out_d[b, :, t], src)

    nc.finalize()
    return nc


# =====================================================================
# host-side preparation
# =====================================================================

def _fold_bn(g, b, m, v):
    s = (np.asarray(g, np.float32) / np.sqrt(np.asarray(v, np.float32) + EPS))
    return (s.astype(np.float32),
            (np.asarray(b, np.float32) - np.asarray(m, np.float32) * s).astype(np.float32))


def prep_inputs(inp):
    f32 = np.float32
    x = np.asarray(inp["x"], f32)

    s_ke, b_ke = _fold_bn(inp["ke_g"], inp["ke_b"], inp["ke_m"], inp["ke_v"])
    s_e1, b_e1 = _fold_bn(inp["e1_g"], inp["e1_b"], inp["e1_m"], inp["e1_v"])
    s_c1, b_c1 = _fold_bn(inp["c1_g"], inp["c1_b"], inp["c1_m"], inp["c1_v"])
    s_b2, b_b2 = _fold_bn(inp["bn2_g"], inp["bn2_b"], inp["bn2_m"], inp["bn2_v"])
    s_s1, b_s1 = _fold_bn(inp["se1_g"], inp["se1_b"], inp["se1_m"], inp["se1_v"])

    kew = np.zeros((27, DIM, DIM), f32)
    KW = np.asarray(inp["ke_w"], f32) * s_ke[:, None, None, None, None]
    for kt in range(K):
        for kh in range(K):
            for kw_ in range(K):
                tap = (kt * K + kh) * K + kw_
                for g in range(4):
                    blk = KW[g * 32:(g + 1) * 32, :, kt, kh, kw_]
                    kew[tap, g * 32:(g + 1) * 32, g * 32:(g + 1) * 32] = blk.T
    kew = kew.transpose(1, 0, 2).reshape(DIM, 27 * DIM).copy()

    E1 = np.asarray(inp["e1_w"], f32) * s_e1[:, None, None, None, None]
    e1w = np.zeros((54, DIM, 64), f32)
    for kt in range(K):
        for kh in range(K):
            for kw_ in range(K):
                tap = (kt * K + kh) * K + kw_
                e1w[tap * 2] = E1[:, :DIM, kt, kh, kw_].T
                e1w[tap * 2 + 1] = E1[:, DIM:, kt, kh, kw_].T
    e1w = e1w.transpose(1, 0, 2).reshape(DIM, 54 * 64).copy()

    # e2 weights packed for frame-stacked moving operands:
    #  A:  [e1[f-1]; e1[f]] pairs -> rows 0:64 = kt0, rows 64:128 = kt1
    #  A1: kt1 only (first frame of a batch has no e1[f-1])
    #  B:  [e1[f+1]; e1[f+1]+WP] pairs -> kt2 with kh0 / kh1
    #  C:  kt2, kh2 singles
    E2 = np.asarray(inp["e2_w"], f32)
    e2wA = np.zeros((DIM, 9 * EMB), f32)
    e2wA1 = np.zeros((64, 9 * EMB), f32)
    for kh in range(K):
        for kw_ in range(K):
            idx = kh * K + kw_
            e2wA[0:64, idx * EMB:(idx + 1) * EMB] = E2[:, :, 0, kh, kw_].T
            e2wA[64:128, idx * EMB:(idx + 1) * EMB] = E2[:, :, 1, kh, kw_].T
            e2wA1[:, idx * EMB:(idx + 1) * EMB] = E2[:, :, 1, kh, kw_].T
    e2wB = np.zeros((DIM, 3 * EMB), f32)
    e2wC = np.zeros((64, 3 * EMB), f32)
    for kw_ in range(K):
        e2wB[0:64, kw_ * EMB:(kw_ + 1) * EMB] = E2[:, :, 2, 0, kw_].T
        e2wB[64:128, kw_ * EMB:(kw_ + 1) * EMB] = E2[:, :, 2, 1, kw_].T
        e2wC[:, kw_ * EMB:(kw_ + 1) * EMB] = E2[:, :, 2, 2, kw_].T

    C1 = np.asarray(inp["c1_w"], f32) * s_c1[:, None, None, None, None]
    c1w = np.zeros((27, DIM, DIM), f32)
    for kt in range(K):
        for kh in range(K):
            for kw_ in range(K):
                tap = (kt * K + kh) * K + kw_
                c1w[tap] = C1[:, :, kt, kh, kw_].T
    c1w = c1w.transpose(1, 0, 2).reshape(DIM, 27 * DIM).copy()

    S1 = np.asarray(inp["se1_w"], f32)[:, :, :, 1, 1] * s_s1[:, None, None]
    se1w = np.zeros((DIM, 3 * 64), f32)
    for kt in range(K):
        se1w[:, kt * 64:(kt + 1) * 64] = (S1[:, :, kt] / (H * W)).T
    se1b = (np.asarray(inp["se1_bias"], f32) * s_s1 + b_s1).reshape(64, 1)

    S2 = np.asarray(inp["se2_w"], f32)[:, :, :, 1, 1]
    se2w = np.zeros((64, 3 * 256), f32)
    for kt in range(K):
        se2w[:, kt * 256:kt * 256 + DIM] = S2[0::2, :, kt].T
        se2w[:, kt * 256 + DIM:kt * 256 + 256] = S2[1::2, :, kt].T
    se2b = np.asarray(inp["se2_bias"], f32)
    se2bd = (se2b[0::2] - se2b[1::2]).reshape(DIM, 1)

    Emat = np.zeros((16, DIM), f32)
    for c in range(DIM):
        Emat[c // SHARE, c] = 1.0
    Gmat = np.zeros((DIM, 16), f32)
    for ch in range(DIM):
        Gmat[ch, ch // 9] = 1.0
    Ghmat = np.zeros((16, 16), f32)
    for i_ in range(16):
        Ghmat[i_, (DIM + i_) // 9] = 1.0

    import ml_dtypes
    bf16 = ml_dtypes.bfloat16
    shared = dict(
        kew=kew.astype(bf16), keb=b_ke.reshape(DIM, 1),
        e1w=e1w.astype(bf16), e1b=b_e1.reshape(64, 1),
        e2wA=e2wA.astype(bf16), e2wA1=e2wA1.astype(bf16),
        e2wB=e2wB.astype(bf16), e2wC=e2wC.astype(bf16),
        e2b=np.asarray(inp["e2_bias"], f32).reshape(EMB, 1),
        c1w=c1w.astype(bf16), c1b=b_c1.reshape(DIM, 1),
        bn2s=np.stack([s_b2, b_b2], axis=1).astype(f32),
        se1w=se1w, se1b=se1b, se2w=se2w, se2bd=se2bd,
        gng=np.asarray(inp["gn_g"], f32).reshape(16, 9),
        gnb=np.asarray(inp["gn_b"], f32).reshape(16, 9),
        Emat=Emat.astype(bf16), Ematf=Emat, Gmat=Gmat, Ghmat=Ghmat,
    )

    def mk_mask(parts, rows, base, dt=f32):
        m = np.zeros((parts, rows, WP), f32)
        for i in range(rows):
            if 0 <= base + i < H:
                m[:, i, 1:97] = 1.0
        return m.reshape(parts, rows * WP).astype(dt)

    per_core = []
    for c in range(NCORES):
        own0 = c * ROWN
        xc = np.zeros((F, DIM, RX, WP), f32)
        r0 = own0 - 3
        lo, hi = max(0, r0), min(H, r0 + RX)
        for b in range(B):
            xc[b * T:(b + 1) * T, :, lo - r0:hi - r0, 1:97] = \
                x[b].transpose(1, 0, 2, 3)[:, :, lo:hi, :]
        d = dict(shared)
        d["xin"] = xc.reshape(F, DIM, LX).astype(bf16)
        d["kmask"] = mk_mask(DIM, RKF, own0 - 2, bf16)
        d["e1mask"] = mk_mask(64, RE1, own0 - 1, bf16)
        d["wvmask"] = mk_mask(DIM, RWV, own0)
        d["xqmask"] = mk_mask(DIM, RXQ, own0 - 1, bf16)
        per_core.append(d)
    return per_core


# =====================================================================
# resident runner
# =====================================================================
class _Runner:
    def __init__(self, nc, n_cores):
        import jax
        from jax.sharding import Mesh, PartitionSpec, NamedSharding
        from jax.experimental.shard_map import shard_map
        from concourse.bass2jax import (_bass_exec_p, install_neuronx_cc_hook,
                                        partition_id_tensor)
        install_neuronx_cc_hook()
        self.jax = jax
        self.nc = nc
        self.n_cores = n_cores

        in_names, out_names, out_avals, zero_outs = [], [], [], []
        pid_name = nc.partition_id_tensor.name if nc.partition_id_tensor else None
        for alloc in nc.m.functions[0].allocations:
            if not isinstance(alloc, mybir.MemoryLocationSet):
                continue
            name = alloc.memorylocations[0].name
            if alloc.kind == "ExternalInput":
                if name != pid_name:
                    in_names.append(name)
            elif alloc.kind == "ExternalOutput":
                out_names.append(name)
                out_avals.append(jax.core.ShapedArray(
                    tuple(alloc.tensor_shape), mybir.dt.np(alloc.dtype)))
                zero_outs.append(np.zeros(tuple(alloc.tensor_shape),
                                          mybir.dt.np(alloc.dtype)))
        self.in_names, self.out_names = in_names, out_names
        self.out_avals, self.zero_outs = out_avals, zero_outs
        n_params = len(in_names)
        all_in_names = list(in_names) + list(out_names)
        if pid_name is not None:
            all_in_names.append(pid_name)
        has_pid = pid_name is not None

        def _body(*args):
            operands = list(args)
            if has_pid:
                operands.append(partition_id_tensor())
            return tuple(_bass_exec_p.bind(
                *operands,
                out_avals=tuple(out_avals),
                in_names=tuple(all_in_names),
                out_names=tuple(out_names),
                lowering_input_output_aliases=(),
                sim_require_finite=True,
                sim_require_nnan=True,
                nc=nc,
            ))

        devices = jax.devices()[:n_cores]
        self.mesh = Mesh(np.asarray(devices), ("core",))
        self.sharding = NamedSharding(self.mesh, PartitionSpec("core"))
        in_specs = (PartitionSpec("core"),) * (n_params + len(out_names))
        out_specs = (PartitionSpec("core"),) * len(out_names)
        self.fn = jax.jit(
            shard_map(_body, mesh=self.mesh, in_specs=in_specs,
                      out_specs=out_specs, check_rep=False),
            keep_unused=True)
        self._zero_dev = None

    def put_inputs(self, in_maps):
        jax = self.jax
        concat = [np.concatenate([np.asarray(in_maps[c][n])
                                  for c in range(self.n_cores)], axis=0)
                  for n in self.in_names]
        if self._zero_dev is None:
            zeros = [np.concatenate([z] * self.n_cores, axis=0)
                     for z in self.zero_outs]
            self._zero_dev = [jax.device_put(z, self.sharding) for z in zeros]
        self._dev_in = [jax.device_put(a, self.sharding) for a in concat]
        jax.block_until_ready(self._dev_in)

    def run(self):
        return self.fn(*self._dev_in, *self._zero_dev)

    def run_np(self):
        jax = self.jax
        out = jax.block_until_ready(self.run())
        res = []
        for c in range(self.n_cores):
            d = {}
            for i, name in enumerate(self.out_names):
                full = np.asarray(out[i])
                d[name] = full.reshape(self.n_cores, *self.out_avals[i].shape)[c]
            res.append(d)
        return res


_CACHE = {}


def _get_runner(debug=False):
    key = ("runner", debug)
    if key not in _CACHE:
        nc = build_nc(debug=debug)
        _CACHE[key] = _Runner(nc, NCORES)
    return _CACHE[key]


def kernel(**inputs) -> np.ndarray:
    per_core = prep_inputs(inputs)
    r = _get_runner(debug=False)
    r.put_inputs(per_core)
    res = r.run_np()
    out = np.concatenate([res[c]["out"] for c in range(NCORES)], axis=3)
    return out.astype(np.float32)

